# revision 1
# baseline (speedup 1.0000x reference)
"""Trainium2 Bass kernel for nn_Contextual_MFN (Memory Fusion Network).

Structure (per core; batch data-parallel 8 ways, 32 rows/core):
  phase 0: xWb[t] = Wih_aug @ x_aug[t]  (all t, fp32r matmuls, bias folded in)
  phase 1: sequential 3xLSTM recurrence; gates = xWb (identity-inject) + Whh@h
  phase 2a: time-parallel attention: att1 MLP -> exp -> U = E*cStar (unnormalized),
            S = sum(E), att2/g1/g2 linear parts on U, bias*S folds
  recip:   Sinv = 1/S
  phase 3: sequential memory-gate recurrence (mem-dependent matmuls only)
  phase 4: output MLP on [h_l, h_a, h_v, mem]

All activations feature-major: [features(partitions), batch(free)].
"""
import os
import numpy as np

import concourse.bass as bass
import concourse.tile as tile
from concourse import bacc, mybir
from concourse.bass_utils import run_bass_kernel_spmd

F32 = mybir.dt.float32
USE_F32R = True
F32R = mybir.dt.float32r
AF = mybir.ActivationFunctionType

# Problem constants (hardcoded; kernel.py must be self-contained)
T_FULL = 512
NBATCH = 256
NCORES = 8
B = NBATCH // NCORES          # 32 batch rows per core
D_L, D_A, D_V = 300, 74, 35
DIN = D_L + D_A + D_V         # 409
DAUG = DIN + 1                # 410 (ones row for bias)
DH = 128
MEM = 256
CH0 = 16                      # phase-0 chunk (steps)
CH2 = 8                       # phase-2a / phase-3 chunk (steps)

# gate slot order: s = g'*3 + m, with g' in (i, f, o, g_tanh); torch rows are (i, f, g, o)
TORCH_G = (0, 1, 3, 2)        # our slot g' -> torch gate row block


def _nonzero_kcs(s):
    """Phase-0 K-chunks (of Waug rows 0..409 padded to 512) that are nonzero for
    output slot s. m=0 (l): feats 0-299 -> kc 0,1,2 (+ones kc3). m=1 (a): 300-373
    -> kc2 (+kc3 ones). m=2 (v): 374-408 -> kc2,kc3 (+ones kc3)."""
    m = s % 3
    if m == 0:
        return [0, 1, 2, 3]
    return [2, 3]


def build_program(Tp=T_FULL):
    global F32R
    F32R = mybir.dt.float32r if USE_F32R else F32
    assert Tp % CH0 == 0 and Tp % CH2 == 0
    NCH2 = Tp // CH2
    nc = bacc.Bacc("TRN2", target_bir_lowering=False, debug=False)

    # ---------------- external inputs ----------------
    xT = nc.dram_tensor("xT", [DAUG, Tp * B], F32, kind="ExternalInput")
    waug = nc.dram_tensor("waug", [512, 1536], F32, kind="ExternalInput")
    whhT = nc.dram_tensor("whhT", [128, 1536], F32, kind="ExternalInput")
    ident = nc.dram_tensor("ident", [128, 128], F32, kind="ExternalInput")
    ones128 = nc.dram_tensor("ones128", [128, 1], F32, kind="ExternalInput")

    a1w1 = nc.dram_tensor("a1w1", [768, 256], F32, kind="ExternalInput")
    a1b1 = nc.dram_tensor("a1b1", [128, 2], F32, kind="ExternalInput")
    a1w2 = nc.dram_tensor("a1w2", [256, 768], F32, kind="ExternalInput")
    a1b2 = nc.dram_tensor("a1b2", [128, 6], F32, kind="ExternalInput")
    a2w1 = nc.dram_tensor("a2w1", [768, 256], F32, kind="ExternalInput")
    a2b1r = nc.dram_tensor("a2b1r", [1, 256], F32, kind="ExternalInput")
    a2w2 = nc.dram_tensor("a2w2", [256, 256], F32, kind="ExternalInput")
    a2b2r = nc.dram_tensor("a2b2r", [1, 256], F32, kind="ExternalInput")
    g1a = nc.dram_tensor("g1a", [768, 256], F32, kind="ExternalInput")
    g2a = nc.dram_tensor("g2a", [768, 256], F32, kind="ExternalInput")
    g1b = nc.dram_tensor("g1b", [256, 256], F32, kind="ExternalInput")
    g2b = nc.dram_tensor("g2b", [256, 256], F32, kind="ExternalInput")
    g1b1r = nc.dram_tensor("g1b1r", [1, 256], F32, kind="ExternalInput")
    g2b1r = nc.dram_tensor("g2b1r", [1, 256], F32, kind="ExternalInput")
    g1w2 = nc.dram_tensor("g1w2", [256, 256], F32, kind="ExternalInput")
    g2w2 = nc.dram_tensor("g2w2", [256, 256], F32, kind="ExternalInput")
    gb2r = nc.dram_tensor("gb2r", [1, 512], F32, kind="ExternalInput")
    ow1 = nc.dram_tensor("ow1", [640, 256], F32, kind="ExternalInput")
    ob1 = nc.dram_tensor("ob1", [128, 2], F32, kind="ExternalInput")
    ow2 = nc.dram_tensor("ow2", [256, 1], F32, kind="ExternalInput")
    ob2 = nc.dram_tensor("ob2", [1, 1], F32, kind="ExternalInput")

    out_d = nc.dram_tensor("out", [B, 1], F32, kind="ExternalOutput")

    # ---------------- internal dram scratch ----------------
    xwb = nc.dram_tensor("xwb", [12, Tp, 128, B], F32)
    cs = [nc.dram_tensor(f"cseq{m}", [Tp + 1, 128, B], F32) for m in range(3)]
    a2r_d = nc.dram_tensor("a2r_d", [NCH2, 128, 2 * CH2 * B], F32)
    g1p_d = nc.dram_tensor("g1p_d", [NCH2, 128, 2 * CH2 * B], F32)
    g2p_d = nc.dram_tensor("g2p_d", [NCH2, 128, 2 * CH2 * B], F32)
    s_d = nc.dram_tensor("s_d", [NCH2, CH2 * B], F32)
    sinv_d = nc.dram_tensor("sinv_d", [NCH2, CH2 * B], F32)

    NB2 = CH2 * B  # 256: phase-2a matmul free dim

    import contextlib
    with tile.TileContext(nc) as tc:
        ctx = contextlib.ExitStack()
        with ctx:
            wpool = ctx.enter_context(tc.tile_pool(name="weights", bufs=1))
            hpool = ctx.enter_context(tc.tile_pool(name="hstate", bufs=2))

            # ---- resident weights / constants in SBUF ----
            wihT_t = wpool.tile([128, 4, 1536], F32R)
            nc.sync.dma_start(
                wihT_t[:], waug.ap().rearrange("(kc p) c -> p kc c", p=128).bitcast(F32R))
            whhT_t = wpool.tile([128, 1536], F32)
            nc.sync.dma_start(whhT_t[:], whhT.ap())
            id_t = wpool.tile([128, 128], F32R)
            nc.sync.dma_start(id_t[:], ident.ap().bitcast(F32R))
            ones128_t = wpool.tile([128, 1], F32R)
            nc.sync.dma_start(ones128_t[:], ones128.ap().bitcast(F32R))

            a1w1_t = wpool.tile([128, 6, 256], F32R)
            nc.sync.dma_start(a1w1_t[:], a1w1.ap().rearrange("(kc p) c -> p kc c", p=128).bitcast(F32R))
            a1b1_t = wpool.tile([128, 2], F32)
            nc.sync.dma_start(a1b1_t[:], a1b1.ap())
            a1w2_t = wpool.tile([128, 2, 768], F32R)
            nc.sync.dma_start(a1w2_t[:], a1w2.ap().rearrange("(kc p) c -> p kc c", p=128).bitcast(F32R))
            a1b2_t = wpool.tile([128, 6], F32)
            nc.sync.dma_start(a1b2_t[:], a1b2.ap())
            a2w1_t = wpool.tile([128, 6, 256], F32R)
            nc.sync.dma_start(a2w1_t[:], a2w1.ap().rearrange("(kc p) c -> p kc c", p=128).bitcast(F32R))
            a2b1r_t = wpool.tile([1, 256], F32R)
            nc.sync.dma_start(a2b1r_t[:], a2b1r.ap().bitcast(F32R))
            a2w2_t = wpool.tile([128, 2, 256], F32R)
            nc.sync.dma_start(a2w2_t[:], a2w2.ap().rearrange("(kc p) c -> p kc c", p=128).bitcast(F32R))
            a2b2r_t = wpool.tile([1, 256], F32R)
            nc.sync.dma_start(a2b2r_t[:], a2b2r.ap().bitcast(F32R))
            g1a_t = wpool.tile([128, 6, 256], F32R)
            nc.sync.dma_start(g1a_t[:], g1a.ap().rearrange("(kc p) c -> p kc c", p=128).bitcast(F32R))
            g2a_t = wpool.tile([128, 6, 256], F32R)
            nc.sync.dma_start(g2a_t[:], g2a.ap().rearrange("(kc p) c -> p kc c", p=128).bitcast(F32R))
            g1b1r_t = wpool.tile([1, 256], F32R)
            nc.sync.dma_start(g1b1r_t[:], g1b1r.ap().bitcast(F32R))
            g2b1r_t = wpool.tile([1, 256], F32R)
            nc.sync.dma_start(g2b1r_t[:], g2b1r.ap().bitcast(F32R))
            g1b_t = wpool.tile([128, 2, 256], F32)
            nc.sync.dma_start(g1b_t[:], g1b.ap().rearrange("(kc p) c -> p kc c", p=128))
            g2b_t = wpool.tile([128, 2, 256], F32)
            nc.sync.dma_start(g2b_t[:], g2b.ap().rearrange("(kc p) c -> p kc c", p=128))
            g1w2_t = wpool.tile([128, 2, 256], F32)
            nc.sync.dma_start(g1w2_t[:], g1w2.ap().rearrange("(kc p) c -> p kc c", p=128))
            g2w2_t = wpool.tile([128, 2, 256], F32)
            nc.sync.dma_start(g2w2_t[:], g2w2.ap().rearrange("(kc p) c -> p kc c", p=128))
            gb2r_t = wpool.tile([1, 512], F32)
            nc.sync.dma_start(gb2r_t[:], gb2r.ap())
            ow1_t = wpool.tile([128, 5, 256], F32)
            nc.sync.dma_start(ow1_t[:], ow1.ap().rearrange("(kc p) c -> p kc c", p=128))
            ob1_t = wpool.tile([128, 2], F32)
            nc.sync.dma_start(ob1_t[:], ob1.ap())
            ow2_t = wpool.tile([128, 2, 1], F32)
            nc.sync.dma_start(ow2_t[:], ow2.ap().rearrange("(kc p) c -> p kc c", p=128))
            ob2_t = wpool.tile([1, 1], F32)
            nc.sync.dma_start(ob2_t[:], ob2.ap())

            ones32_t = wpool.tile([1, 32], F32)
            nc.vector.memset(ones32_t[:], 1.0)
            ones1x128_t = wpool.tile([1, 128], F32)
            nc.vector.memset(ones1x128_t[:], 1.0)
            zero_t = wpool.tile([128, 32], F32)
            nc.vector.memset(zero_t[:], 0.0)

            # =================== PHASE 0: xWb ===================
            with (
                tc.tile_pool(name="p0x", bufs=2) as p0x,
                tc.tile_pool(name="p0s", bufs=4) as p0s,
                tc.tile_pool(name="p0p", bufs=4, space="PSUM") as p0p,
            ):
                for k0 in range(Tp // CH0):
                    t0 = k0 * CH0
                    n0 = CH0 * B  # 512
                    xt = p0x.tile([128, 4, n0], F32R, tag="xt")
                    for kc in range(4):
                        rows = 128 if kc < 3 else DAUG - 384  # 26 on last chunk
                        nc.sync.dma_start(
                            xt[0:rows, kc, :],
                            xT.ap()[kc * 128:kc * 128 + rows, t0 * B:(t0 + CH0) * B].bitcast(F32R))
                    for s in range(12):
                        pt = p0p.tile([128, n0], F32, tag="p0acc")
                        kcs = _nonzero_kcs(s)
                        for i, kc in enumerate(kcs):
                            rows = 128 if kc < 3 else DAUG - 384
                            nc.tensor.matmul(
                                pt[:], wihT_t[0:rows, kc, s * 128:(s + 1) * 128],
                                xt[0:rows, kc, :],
                                start=(i == 0), stop=(i == len(kcs) - 1))
                        st = p0s.tile([128, n0], F32, tag="p0st")
                        if s % 2 == 0:
                            nc.vector.tensor_copy(st[:], pt[:])
                        else:
                            nc.scalar.copy(st[:], pt[:])
                        # dram [CH0, 128, B] slab, partition-major write
                        nc.sync.dma_start(
                            xwb.ap()[s, t0:t0 + CH0, :, :].transpose([1, 0, 2]),
                            st[:].rearrange("p (t b) -> p t b", b=B))

            tc.strict_bb_all_engine_barrier()

            # =================== PHASE 1: LSTM recurrence ===================
            with (
                tc.tile_pool(name="p1w", bufs=2) as p1w,
                tc.tile_pool(name="p1s", bufs=3) as p1s,
                tc.tile_pool(name="p1c", bufs=4) as p1c,
                tc.tile_pool(name="p1p", bufs=2, space="PSUM") as p1p,
            ):
                h_cur = hpool.tile([128, 96], F32, tag="h")
                nc.vector.memset(h_cur[:], 0.0)
                c_cur = p1c.tile([128, 3, 32], F32, tag="c")
                nc.vector.memset(c_cur[:], 0.0)
                for m in range(3):
                    nc.sync.dma_start(cs[m].ap()[0], zero_t[:])

                h_fin = None
                for w in range(Tp // CH2):
                    t0 = w * CH2
                    win = p1w.tile([128, 12, CH2, 32], F32R, tag="xwbwin")
                    for s in range(12):
                        nc.sync.dma_start(
                            win[:, s, :, :],
                            xwb.ap()[s, t0:t0 + CH2, :, :].transpose([1, 0, 2]).bitcast(F32R))
                    for j in range(CH2):
                        t = t0 + j
                        gp = p1p.tile([128, 12, 32], F32, tag="gates")
                        nc.tensor.matmul(gp[:], id_t[:], win[:, :, j, :],
                                         start=True, stop=False)
                        for s in range(12):
                            gq, m = divmod(s, 3)
                            nc.tensor.matmul(
                                gp[:, s, :],
                                whhT_t[:, s * 128:(s + 1) * 128],
                                h_cur[:, m * 32:(m + 1) * 32],
                                start=False, stop=(s == 11))
                        sg = p1s.tile([128, 9, 32], F32, tag="sg")
                        nc.scalar.activation(sg[:], gp[:, 0:9, :], AF.Sigmoid)
                        tg = p1s.tile([128, 3, 32], F32, tag="tg")
                        nc.scalar.activation(tg[:], gp[:, 9:12, :], AF.Tanh)
                        t1 = p1s.tile([128, 3, 32], F32, tag="t1")
                        nc.vector.tensor_mul(t1[:], sg[:, 0:3, :], tg[:])
                        t2 = p1s.tile([128, 3, 32], F32, tag="t2")
                        nc.vector.tensor_mul(t2[:], sg[:, 3:6, :], c_cur[:])
                        c_new = p1c.tile([128, 3, 32], F32, tag="c")
                        nc.vector.tensor_add(c_new[:], t1[:], t2[:])
                        tc_t = p1s.tile([128, 3, 32], F32, tag="tc")
                        nc.scalar.activation(tc_t[:], c_new[:], AF.Tanh)
                        h_new = hpool.tile([128, 96], F32, tag="h")
                        nc.vector.tensor_mul(
                            h_new[:].rearrange("p (m b) -> p m b", b=32),
                            sg[:, 6:9, :], tc_t[:])
                        for m in range(3):
                            nc.sync.dma_start(cs[m].ap()[t + 1], c_new[:, m, :])
                        c_cur = c_new
                        h_cur = h_new
                h_fin = h_cur

            tc.strict_bb_all_engine_barrier()

            # =================== PHASE 2a: time-parallel attention ===================
            with (
                tc.tile_pool(name="p2c", bufs=2) as p2c,
                tc.tile_pool(name="p2s", bufs=2) as p2s,
                tc.tile_pool(name="p2r", bufs=3) as p2r,
                tc.tile_pool(name="p2p1", bufs=2, space="PSUM") as p2p1,
                tc.tile_pool(name="p2pe", bufs=1, space="PSUM") as p2pe,
                tc.tile_pool(name="p2po", bufs=2, space="PSUM") as p2po,
                tc.tile_pool(name="p2ps", bufs=1, space="PSUM") as p2ps,
            ):
                for k in range(NCH2):
                    t0 = k * CH2
                    cw = [p2c.tile([128, CH2 + 1, 32], F32R, tag=f"cw{m}", name=f"cw{m}")
                          for m in range(3)]
                    for m in range(3):
                        nc.sync.dma_start(
                            cw[m][:], cs[m].ap()[t0:t0 + CH2 + 1].transpose([1, 0, 2]).bitcast(F32R))

                    def rhs_k(kc):
                        if kc < 3:
                            return cw[kc][:, 0:CH2, :]
                        return cw[kc - 3][:, 1:CH2 + 1, :]

                    # att1 layer 1 + relu
                    y1p = p2p1.tile([128, 2, NB2], F32, tag="stage1")
                    for mc in range(2):
                        for kc in range(6):
                            nc.tensor.matmul(
                                y1p[:, mc, :], a1w1_t[:, kc, mc * 128:(mc + 1) * 128],
                                rhs_k(kc), start=(kc == 0), stop=(kc == 5))
                    y1 = p2s.tile([128, 2, NB2], F32R, tag="y1")
                    for mc in range(2):
                        nc.scalar.activation(y1[:, mc, :], y1p[:, mc, :], AF.Relu,
                                             bias=a1b1_t[:, mc:mc + 1])
                    # att1 layer 2 + exp
                    ep = p2pe.tile([128, 6, NB2], F32, tag="logits")
                    for mc6 in range(6):
                        for kc in range(2):
                            nc.tensor.matmul(
                                ep[:, mc6, :], a1w2_t[:, kc, mc6 * 128:(mc6 + 1) * 128],
                                y1[:, kc, :], start=(kc == 0), stop=(kc == 1))
                    et = p2s.tile([128, 6, NB2], F32R, tag="et")
                    for mc6 in range(6):
                        nc.scalar.activation(et[:, mc6, :], ep[:, mc6, :], AF.Exp,
                                             bias=a1b2_t[:, mc6:mc6 + 1])
                    # U = E * cStar (unnormalized attended)
                    ut = p2s.tile([128, 6, NB2], F32R, tag="ut")
                    for q in range(6):
                        nc.vector.tensor_mul(ut[:, q, :], et[:, q, :].bitcast(F32),
                                             rhs_k(q).bitcast(F32))
                    # S = sum over features of E
                    sp = p2ps.tile([1, NB2], F32, tag="srow")
                    for q in range(6):
                        nc.tensor.matmul(sp[:], ones128_t[:], et[:, q, :],
                                         start=(q == 0), stop=(q == 5))
                    srow = p2r.tile([1, NB2], F32R, tag="srow_s")
                    nc.vector.tensor_copy(srow[:], sp[:])
                    nc.sync.dma_start(s_d.ap()[k:k + 1, :], srow[:].bitcast(F32))

                    # att2 layer 1 + relu
                    zp = p2p1.tile([128, 2, NB2], F32, tag="stage1")
                    for mc in range(2):
                        for kc in range(6):
                            nc.tensor.matmul(
                                zp[:, mc, :], a2w1_t[:, kc, mc * 128:(mc + 1) * 128],
                                ut[:, kc, :], start=(kc == 0), stop=False)
                        nc.tensor.matmul(zp[:, mc, :], a2b1r_t[:, mc * 128:(mc + 1) * 128],
                                         srow[:], start=False, stop=True)
                    z = p2s.tile([128, 2, NB2], F32R, tag="z")
                    for mc in range(2):
                        nc.scalar.activation(z[:, mc, :], zp[:, mc, :], AF.Relu)
                    # att2 layer 2 (raw) + b2*S fold
                    ap2 = p2po.tile([128, 2, NB2], F32, tag="out")
                    for mc in range(2):
                        for kc in range(2):
                            nc.tensor.matmul(
                                ap2[:, mc, :], a2w2_t[:, kc, mc * 128:(mc + 1) * 128],
                                z[:, kc, :], start=(kc == 0), stop=False)
                        nc.tensor.matmul(ap2[:, mc, :], a2b2r_t[:, mc * 128:(mc + 1) * 128],
                                         srow[:], start=False, stop=True)
                    a2s = p2s.tile([128, 2, NB2], F32, tag="a2s")
                    nc.scalar.copy(a2s[:], ap2[:])
                    nc.sync.dma_start(a2r_d.ap()[k], a2s[:].rearrange("p a b -> p (a b)"))

                    # g1 / g2 attended-part + b1*S fold
                    for gi, (gw, gbr, gd) in enumerate(
                            ((g1a_t, g1b1r_t, g1p_d), (g2a_t, g2b1r_t, g2p_d))):
                        gp2 = p2po.tile([128, 2, NB2], F32, tag="out")
                        for mc in range(2):
                            for kc in range(6):
                                nc.tensor.matmul(
                                    gp2[:, mc, :], gw[:, kc, mc * 128:(mc + 1) * 128],
                                    ut[:, kc, :], start=(kc == 0), stop=False)
                            nc.tensor.matmul(gp2[:, mc, :], gbr[:, mc * 128:(mc + 1) * 128],
                                             srow[:], start=False, stop=True)
                        gs = p2s.tile([128, 2, NB2], F32, tag=f"g{gi}s")
                        if gi == 0:
                            nc.vector.tensor_copy(gs[:], gp2[:])
                        else:
                            nc.scalar.copy(gs[:], gp2[:])
                        nc.sync.dma_start(gd.ap()[k], gs[:].rearrange("p a b -> p (a b)"))

            tc.strict_bb_all_engine_barrier()

            # =================== reciprocal of S ===================
            with tc.tile_pool(name="prc", bufs=1) as prc:
                nrows = NCH2
                sall = prc.tile([nrows, NB2], F32)
                nc.sync.dma_start(sall[:], s_d.ap())
                sinv = prc.tile([nrows, NB2], F32)
                nc.vector.reciprocal(sinv[:], sall[:])
                nc.sync.dma_start(sinv_d.ap(), sinv[:])

            tc.strict_bb_all_engine_barrier()

            # =================== PHASE 3: memory recurrence ===================
            with (
                tc.tile_pool(name="p3w", bufs=2) as p3w,
                tc.tile_pool(name="p3s", bufs=3) as p3s,
                tc.tile_pool(name="p3m", bufs=2) as p3m,
                tc.tile_pool(name="p3p", bufs=2, space="PSUM") as p3p,
                tc.tile_pool(name="p3pb", bufs=2, space="PSUM") as p3pb,
            ):
                mem_cur = p3m.tile([128, 2, 32], F32, tag="mem")
                nc.vector.memset(mem_cur[:], 0.0)
                for k in range(NCH2):
                    aw = p3w.tile([128, 2, CH2, 32], F32, tag="aw")
                    nc.sync.dma_start(aw[:], a2r_d.ap()[k].rearrange("p (a t b) -> p a t b", a=2, b=32))
                    g1w_ = p3w.tile([128, 2, CH2, 32], F32, tag="g1w")
                    nc.sync.dma_start(g1w_[:], g1p_d.ap()[k].rearrange("p (a t b) -> p a t b", a=2, b=32))
                    g2w_ = p3w.tile([128, 2, CH2, 32], F32, tag="g2w")
                    nc.sync.dma_start(g2w_[:], g2p_d.ap()[k].rearrange("p (a t b) -> p a t b", a=2, b=32))
                    sr = p3w.tile([1, NB2], F32, tag="sr")
                    nc.sync.dma_start(sr[:], sinv_d.ap()[k:k + 1, :])

                    for j in range(CH2):
                        # broadcast Sinv_t across partitions via K=1 matmul
                        sb = p3pb.tile([128, 32], F32, tag="sinvb")
                        nc.tensor.matmul(sb[:], ones1x128_t[:], sr[:, j * 32:(j + 1) * 32],
                                         start=True, stop=True)
                        sb2 = sb[:].unsqueeze(1).broadcast_to([128, 2, 32])
                        # normalized g-pre parts
                        u = p3s.tile([128, 4, 32], F32, tag="u")
                        nc.vector.tensor_mul(u[:, 0:2, :], g1w_[:, :, j, :], sb2)
                        nc.vector.tensor_mul(u[:, 2:4, :], g2w_[:, :, j, :], sb2)
                        # mem-part matmuls (+ nothing else: b1*S already folded)
                        pg = p3p.tile([128, 4, 32], F32, tag="gmm")
                        for r, (gwt,) in enumerate(((g1b_t,), (g1b_t,), (g2b_t,), (g2b_t,))):
                            mc = r % 2
                            for kc in range(2):
                                nc.tensor.matmul(
                                    pg[:, r, :], gwt[:, kc, mc * 128:(mc + 1) * 128],
                                    mem_cur[:, kc, :], start=(kc == 0), stop=(kc == 1))
                        w_t = p3s.tile([128, 4, 32], F32, tag="w")
                        nc.vector.tensor_add(w_t[:], u[:], pg[:])
                        hh = p3s.tile([128, 4, 32], F32, tag="hh")
                        nc.scalar.activation(hh[:], w_t[:], AF.Relu)
                        # L2 + b2 fold
                        qg = p3p.tile([128, 4, 32], F32, tag="qmm")
                        for r, gwt in enumerate((g1w2_t, g1w2_t, g2w2_t, g2w2_t)):
                            mc = r % 2
                            goff = 0 if r < 2 else 2
                            for kc in range(2):
                                nc.tensor.matmul(
                                    qg[:, r, :], gwt[:, kc, mc * 128:(mc + 1) * 128],
                                    hh[:, goff + kc, :], start=(kc == 0), stop=False)
                            nc.tensor.matmul(qg[:, r, :], gb2r_t[:, r * 128:(r + 1) * 128],
                                             ones32_t[:], start=False, stop=True)
                        gam = p3s.tile([128, 4, 32], F32, tag="gam")
                        nc.scalar.activation(gam[:], qg[:], AF.Sigmoid)
                        # cHat = tanh(att2raw * Sinv)  (b2*S folded already)
                        v = p3s.tile([128, 2, 32], F32, tag="v")
                        nc.vector.tensor_mul(v[:], aw[:, :, j, :], sb2)
                        ch = p3s.tile([128, 2, 32], F32, tag="ch")
                        nc.scalar.activation(ch[:], v[:], AF.Tanh)
                        # mem = gam1*mem + gam2*cHat
                        m1 = p3s.tile([128, 2, 32], F32, tag="m1")
                        nc.vector.tensor_mul(m1[:], gam[:, 0:2, :], mem_cur[:])
                        m2 = p3s.tile([128, 2, 32], F32, tag="m2")
                        nc.vector.tensor_mul(m2[:], gam[:, 2:4, :], ch[:])
                        mem_new = p3m.tile([128, 2, 32], F32, tag="mem")
                        nc.vector.tensor_add(mem_new[:], m1[:], m2[:])
                        mem_cur = mem_new

                # =================== PHASE 4: output MLP ===================
                with tc.tile_pool(name="p4p", bufs=1, space="PSUM") as p4p:
                    o1p = p4p.tile([128, 2, 32], F32, tag="o1")
                    rhs5 = [h_fin[:, 0:32], h_fin[:, 32:64], h_fin[:, 64:96],
                            mem_cur[:, 0, :], mem_cur[:, 1, :]]
                    for mc in range(2):
                        for kc in range(5):
                            nc.tensor.matmul(
                                o1p[:, mc, :], ow1_t[:, kc, mc * 128:(mc + 1) * 128],
                                rhs5[kc], start=(kc == 0), stop=(kc == 4))
                    o1s = p3s.tile([128, 2, 32], F32, tag="o1s")
                    for mc in range(2):
                        nc.scalar.activation(o1s[:, mc, :], o1p[:, mc, :], AF.Relu,
                                             bias=ob1_t[:, mc:mc + 1])
                    o2p = p4p.tile([1, 32], F32, tag="o2")
                    for kc in range(2):
                        nc.tensor.matmul(o2p[:], ow2_t[:, kc, :], o1s[:, kc, :],
                                         start=(kc == 0), stop=(kc == 1))
                    o2s = p3s.tile([1, 32], F32, tag="o2s")
                    nc.scalar.activation(o2s[:], o2p[:], AF.Identity, bias=ob2_t[:])
                    nc.sync.dma_start(out_d.ap().rearrange("b one -> (one) (b)"), o2s[:])

    nc.compile()
    return nc


# ---------------------------------------------------------------------------
# host-side packing
# ---------------------------------------------------------------------------

def pack_shared(inp):
    """Pack weight tensors (identical across cores)."""
    f = np.float32
    d = {}
    wih = {0: inp["Wih_l"], 1: inp["Wih_a"], 2: inp["Wih_v"]}
    whh = {0: inp["Whh_l"], 1: inp["Whh_a"], 2: inp["Whh_v"]}
    bb = {m: (inp[f"bih_{k}"] + inp[f"bhh_{k}"]).astype(f)
          for m, k in ((0, "l"), (1, "a"), (2, "v"))}
    foff = {0: 0, 1: D_L, 2: D_L + D_A}
    din = {0: D_L, 1: D_A, 2: D_V}

    waug = np.zeros((512, 1536), f)
    whhT = np.zeros((128, 1536), f)
    for gq in range(4):
        tg = TORCH_G[gq]
        for m in range(3):
            s = gq * 3 + m
            wblk = wih[m][tg * 128:(tg + 1) * 128, :]          # [128, din]
            waug[foff[m]:foff[m] + din[m], s * 128:(s + 1) * 128] = wblk.T
            waug[DIN, s * 128:(s + 1) * 128] = bb[m][tg * 128:(tg + 1) * 128]
            whhT[:, s * 128:(s + 1) * 128] = whh[m][tg * 128:(tg + 1) * 128, :].T
    d["waug"] = waug
    d["whhT"] = whhT
    d["ident"] = np.eye(128, dtype=f)
    d["ones128"] = np.ones((128, 1), f)

    d["a1w1"] = inp["att1_W1"].T.astype(f).copy()              # [768, 256]
    d["a1b1"] = inp["att1_b1"].reshape(2, 128).T.astype(f).copy()
    d["a1w2"] = inp["att1_W2"].T.astype(f).copy()              # [256, 768]
    d["a1b2"] = inp["att1_b2"].reshape(6, 128).T.astype(f).copy()
    d["a2w1"] = inp["att2_W1"].T.astype(f).copy()
    d["a2b1r"] = inp["att2_b1"].reshape(1, 256).astype(f).copy()
    d["a2w2"] = inp["att2_W2"].T.astype(f).copy()              # [256, 256]
    d["a2b2r"] = inp["att2_b2"].reshape(1, 256).astype(f).copy()
    d["g1a"] = inp["g1_W1"][:, :768].T.astype(f).copy()
    d["g2a"] = inp["g2_W1"][:, :768].T.astype(f).copy()
    d["g1b"] = inp["g1_W1"][:, 768:].T.astype(f).copy()
    d["g2b"] = inp["g2_W1"][:, 768:].T.astype(f).copy()
    d["g1b1r"] = inp["g1_b1"].reshape(1, 256).astype(f).copy()
    d["g2b1r"] = inp["g2_b1"].reshape(1, 256).astype(f).copy()
    d["g1w2"] = inp["g1_W2"].T.astype(f).copy()
    d["g2w2"] = inp["g2_W2"].T.astype(f).copy()
    d["gb2r"] = np.concatenate([inp["g1_b2"], inp["g2_b2"]]).reshape(1, 512).astype(f)
    d["ow1"] = inp["out_W1"].T.astype(f).copy()                # [640, 256]
    d["ob1"] = inp["out_b1"].reshape(2, 128).T.astype(f).copy()
    d["ow2"] = inp["out_W2"].T.astype(f).copy()                # [256, 1]
    d["ob2"] = inp["out_b2"].reshape(1, 1).astype(f).copy()
    return d


def pack_x(x, core, Tp):
    """x: [Tp, 256, 409] -> xT [410, Tp*B] for one core."""
    xc = np.asarray(x[:, core * B:(core + 1) * B, :], np.float32)   # [Tp, B, 409]
    xt = xc.transpose(2, 0, 1).reshape(DIN, Tp * B)
    return np.concatenate([xt, np.ones((1, Tp * B), np.float32)], 0)


_CACHE = {}


def _get_program(Tp):
    if Tp not in _CACHE:
        _CACHE[Tp] = build_program(Tp)
    return _CACHE[Tp]


def kernel(**inputs):
    x = np.asarray(inputs["x"])
    Tp = x.shape[0]
    nc = _get_program(Tp)
    shared = pack_shared({k: np.asarray(v) for k, v in inputs.items()})
    in_maps = []
    for c in range(NCORES):
        m = dict(shared)
        m["xT"] = np.ascontiguousarray(pack_x(x, c, Tp))
        in_maps.append(m)
    res = run_bass_kernel_spmd(nc, in_maps, list(range(NCORES))).results
    out = np.concatenate([r["out"] for r in res], axis=0)
    return out.astype(np.float32)


if __name__ == "__main__":
    import time
    t0 = time.time()
    nc = build_program(32)
    print("built in", time.time() - t0, "s")



# revision 12
# speedup vs baseline: 2.5301x; 2.5301x over previous
"""Trainium2 Bass kernel for nn_Contextual_MFN (Memory Fusion Network).

v2: all-bf16 matmuls (fp32 PSUM accumulate), normalize-early attention.

Structure (per core; batch data-parallel 8 ways, 32 rows/core):
  phase 0: xWb[t] = Wih_aug @ x_aug[t]  (all t, bf16 matmuls, bias via ones row)
  phase 1: sequential 3xLSTM recurrence; gates = xWb (identity-inject) + Whh@h
  phase 2a: time-parallel attention, NORMALIZED early:
            E = exp(att1(cStar)), S = sum(E), A = E*cStar/S (attended),
            cHat = tanh(att2(A)) fully computed here,
            g-att-parts = W1a@A + b1 (stored; only mem part left for ph3)
  phase 3: sequential memory-gate recurrence (mem-dependent matmuls only)
  phase 4: output MLP on [h_l, h_a, h_v, mem]

All activations feature-major: [features(partitions), batch(free)].
"""
import numpy as np
import ml_dtypes

import concourse.bass as bass
import concourse.tile as tile
from concourse import bacc, mybir
from concourse.bass_utils import run_bass_kernel_spmd

F32 = mybir.dt.float32
BF16 = mybir.dt.bfloat16
AF = mybir.ActivationFunctionType
NPBF = ml_dtypes.bfloat16

# Problem constants (hardcoded; kernel.py must be self-contained)
T_FULL = 512
NBATCH = 256
NCORES = 8
B = NBATCH // NCORES          # 32 batch rows per core
D_L, D_A, D_V = 300, 74, 35
DIN = D_L + D_A + D_V         # 409
DAUG = DIN + 1                # 410 (ones row for bias)
DH = 128
MEM = 256
CH0 = 16                      # phase-0 chunk (steps)
CH2 = 8                       # phase-2a / phase-3 chunk (steps)

# gate slot order: s = g'*3 + m, with g' in (i, f, o, g_tanh); torch rows are (i, f, g, o)
TORCH_G = (0, 1, 3, 2)        # our slot g' -> torch gate row block


def _nonzero_kcs(s):
    """Phase-0 K-chunks (of Waug rows 0..409 padded to 512) that are nonzero for
    output slot s. m=0 (l): feats 0-299 -> kc 0,1,2 (+ones kc3). m=1 (a): 300-373
    -> kc2 (+kc3 ones). m=2 (v): 374-408 -> kc2,kc3 (+ones kc3)."""
    m = s % 3
    if m == 0:
        return [0, 1, 2, 3]
    return [2, 3]


def build_program(Tp=T_FULL):
    assert Tp % CH0 == 0 and Tp % CH2 == 0
    NCH2 = Tp // CH2
    nc = bacc.Bacc("TRN2", target_bir_lowering=False, debug=False)

    # ---------------- external inputs ----------------
    xT = nc.dram_tensor("xT", [DAUG, Tp * B], BF16, kind="ExternalInput")
    waug = nc.dram_tensor("waug", [512, 1536], BF16, kind="ExternalInput")
    whhT = nc.dram_tensor("whhT", [128, 1536], BF16, kind="ExternalInput")
    ident = nc.dram_tensor("ident", [128, 128], BF16, kind="ExternalInput")
    ones128 = nc.dram_tensor("ones128", [128, 1], BF16, kind="ExternalInput")

    a1w1 = nc.dram_tensor("a1w1", [768, 256], BF16, kind="ExternalInput")
    a1b1 = nc.dram_tensor("a1b1", [128, 2], F32, kind="ExternalInput")
    a1w2 = nc.dram_tensor("a1w2", [256, 768], BF16, kind="ExternalInput")
    a1b2 = nc.dram_tensor("a1b2", [128, 6], F32, kind="ExternalInput")
    a2w1 = nc.dram_tensor("a2w1", [768, 256], BF16, kind="ExternalInput")
    a2b1f = nc.dram_tensor("a2b1f", [128, 2], F32, kind="ExternalInput")
    a2w2 = nc.dram_tensor("a2w2", [256, 256], BF16, kind="ExternalInput")
    a2b2f = nc.dram_tensor("a2b2f", [128, 2], F32, kind="ExternalInput")
    g1a = nc.dram_tensor("g1a", [768, 256], BF16, kind="ExternalInput")
    g2a = nc.dram_tensor("g2a", [768, 256], BF16, kind="ExternalInput")
    g1b = nc.dram_tensor("g1b", [256, 256], BF16, kind="ExternalInput")
    g2b = nc.dram_tensor("g2b", [256, 256], BF16, kind="ExternalInput")
    g1b1f = nc.dram_tensor("g1b1f", [128, 2], F32, kind="ExternalInput")
    g2b1f = nc.dram_tensor("g2b1f", [128, 2], F32, kind="ExternalInput")
    g1w2 = nc.dram_tensor("g1w2", [256, 256], BF16, kind="ExternalInput")
    g2w2 = nc.dram_tensor("g2w2", [256, 256], BF16, kind="ExternalInput")
    b2f4 = nc.dram_tensor("b2f4", [4, 128], BF16, kind="ExternalInput")
    e4sel = nc.dram_tensor("e4sel", [4, 128], BF16, kind="ExternalInput")
    ow1 = nc.dram_tensor("ow1", [640, 256], BF16, kind="ExternalInput")
    ob1 = nc.dram_tensor("ob1", [128, 2], F32, kind="ExternalInput")
    ow2 = nc.dram_tensor("ow2", [256, 1], BF16, kind="ExternalInput")
    ob2 = nc.dram_tensor("ob2", [1, 1], F32, kind="ExternalInput")

    out_d = nc.dram_tensor("out", [B, 1], F32, kind="ExternalOutput")

    # ---------------- internal dram scratch ----------------
    # xwb2: per ph1-chunk, partition-major: [chunk, 128, slot, step, batch]
    xwb2 = nc.dram_tensor("xwb2", [NCH2, 128, 12, CH2, B], BF16)
    cs = [nc.dram_tensor(f"cseq{m}", [Tp + 1, 128, B], BF16) for m in range(3)]
    chat_d = nc.dram_tensor("chat_d", [NCH2, 128, 2 * CH2 * B], BF16)
    gp_d = nc.dram_tensor("gp_d", [NCH2, 128, 4 * CH2 * B], BF16)

    NB2 = CH2 * B  # 256: phase-2a matmul free dim

    import contextlib
    with tile.TileContext(nc) as tc:
        ctx = contextlib.ExitStack()
        with ctx:
            wpool = ctx.enter_context(tc.tile_pool(name="weights", bufs=1))
            hpool = ctx.enter_context(tc.tile_pool(name="hstate", bufs=2))

            # ---- resident weights / constants in SBUF ----
            wihT_t = wpool.tile([128, 4, 1536], BF16)
            nc.sync.dma_start(
                wihT_t[:], waug.ap().rearrange("(kc p) c -> p kc c", p=128))
            whhT_t = wpool.tile([128, 1536], BF16)
            nc.sync.dma_start(whhT_t[:], whhT.ap())
            id_t = wpool.tile([128, 128], BF16)
            nc.sync.dma_start(id_t[:], ident.ap())
            ones128_t = wpool.tile([128, 1], BF16)
            nc.sync.dma_start(ones128_t[:], ones128.ap())

            a1w1_t = wpool.tile([128, 6, 256], BF16)
            nc.sync.dma_start(a1w1_t[:], a1w1.ap().rearrange("(kc p) c -> p kc c", p=128))
            a1b1_t = wpool.tile([128, 2], F32)
            nc.sync.dma_start(a1b1_t[:], a1b1.ap())
            a1w2_t = wpool.tile([128, 2, 768], BF16)
            nc.sync.dma_start(a1w2_t[:], a1w2.ap().rearrange("(kc p) c -> p kc c", p=128))
            a1b2_t = wpool.tile([128, 6], F32)
            nc.sync.dma_start(a1b2_t[:], a1b2.ap())
            a2w1_t = wpool.tile([128, 6, 256], BF16)
            nc.sync.dma_start(a2w1_t[:], a2w1.ap().rearrange("(kc p) c -> p kc c", p=128))
            a2b1f_t = wpool.tile([128, 2], F32)
            nc.sync.dma_start(a2b1f_t[:], a2b1f.ap())
            a2w2_t = wpool.tile([128, 2, 256], BF16)
            nc.sync.dma_start(a2w2_t[:], a2w2.ap().rearrange("(kc p) c -> p kc c", p=128))
            a2b2f_t = wpool.tile([128, 2], F32)
            nc.sync.dma_start(a2b2f_t[:], a2b2f.ap())
            g1a_t = wpool.tile([128, 6, 256], BF16)
            nc.sync.dma_start(g1a_t[:], g1a.ap().rearrange("(kc p) c -> p kc c", p=128))
            g2a_t = wpool.tile([128, 6, 256], BF16)
            nc.sync.dma_start(g2a_t[:], g2a.ap().rearrange("(kc p) c -> p kc c", p=128))
            g1b1f_t = wpool.tile([128, 2], F32)
            nc.sync.dma_start(g1b1f_t[:], g1b1f.ap())
            g2b1f_t = wpool.tile([128, 2], F32)
            nc.sync.dma_start(g2b1f_t[:], g2b1f.ap())
            g1b_t = wpool.tile([128, 2, 256], BF16)
            nc.sync.dma_start(g1b_t[:], g1b.ap().rearrange("(kc p) c -> p kc c", p=128))
            g2b_t = wpool.tile([128, 2, 256], BF16)
            nc.sync.dma_start(g2b_t[:], g2b.ap().rearrange("(kc p) c -> p kc c", p=128))
            g1w2_t = wpool.tile([128, 2, 256], BF16)
            nc.sync.dma_start(g1w2_t[:], g1w2.ap().rearrange("(kc p) c -> p kc c", p=128))
            g2w2_t = wpool.tile([128, 2, 256], BF16)
            nc.sync.dma_start(g2w2_t[:], g2w2.ap().rearrange("(kc p) c -> p kc c", p=128))
            b2f4_t = wpool.tile([4, 128], BF16)
            nc.sync.dma_start(b2f4_t[:], b2f4.ap())
            e4sel_t = wpool.tile([4, 128], BF16)
            nc.sync.dma_start(e4sel_t[:], e4sel.ap())
            ow1_t = wpool.tile([128, 5, 256], BF16)
            nc.sync.dma_start(ow1_t[:], ow1.ap().rearrange("(kc p) c -> p kc c", p=128))
            ob1_t = wpool.tile([128, 2], F32)
            nc.sync.dma_start(ob1_t[:], ob1.ap())
            ow2_t = wpool.tile([128, 2, 1], BF16)
            nc.sync.dma_start(ow2_t[:], ow2.ap().rearrange("(kc p) c -> p kc c", p=128))
            ob2_t = wpool.tile([1, 1], F32)
            nc.sync.dma_start(ob2_t[:], ob2.ap())

            ones1x128_t = wpool.tile([1, 128], BF16)
            nc.vector.memset(ones1x128_t[:], 1.0)
            zero_t = wpool.tile([128, 32], BF16)
            nc.vector.memset(zero_t[:], 0.0)

            # =================== PHASE 0: xWb ===================
            with (
                tc.tile_pool(name="p0x", bufs=2) as p0x,
                tc.tile_pool(name="p0s", bufs=6) as p0s,
                tc.tile_pool(name="p0p", bufs=4, space="PSUM") as p0p,
            ):
                for k0 in range(Tp // CH0):
                    t0 = k0 * CH0
                    n0 = CH0 * B  # 512
                    xt = p0x.tile([128, 4, n0], BF16, tag="xt")
                    for kc in range(4):
                        rows = 128 if kc < 3 else DAUG - 384  # 26 on last chunk
                        nc.sync.dma_start(
                            xt[0:rows, kc, :],
                            xT.ap()[kc * 128:kc * 128 + rows, t0 * B:(t0 + CH0) * B])
                    for s in range(12):
                        pt = p0p.tile([128, n0], F32, tag="p0acc")
                        kcs = _nonzero_kcs(s)
                        for i, kc in enumerate(kcs):
                            rows = 128 if kc < 3 else DAUG - 384
                            nc.tensor.matmul(
                                pt[:], wihT_t[0:rows, kc, s * 128:(s + 1) * 128],
                                xt[0:rows, kc, :],
                                start=(i == 0), stop=(i == len(kcs) - 1))
                        st = p0s.tile([128, n0], BF16, tag="p0st")
                        if s % 2 == 0:
                            nc.vector.tensor_copy(st[:], pt[:])
                        else:
                            nc.scalar.copy(st[:], pt[:])
                        # write both ph1-chunks covered by this CH0 slab
                        stv = st[:].rearrange("p (t b) -> p t b", b=B)
                        for h in range(CH0 // CH2):
                            nc.sync.dma_start(
                                xwb2.ap()[k0 * (CH0 // CH2) + h, :, s, :, :],
                                stv[:, h * CH2:(h + 1) * CH2, :])

            tc.strict_bb_all_engine_barrier()

            # =================== PHASE 1: LSTM recurrence ===================
            with (
                tc.tile_pool(name="p1w", bufs=2) as p1w,
                tc.tile_pool(name="p1s", bufs=3) as p1s,
                tc.tile_pool(name="p1c", bufs=4) as p1c,
                tc.tile_pool(name="p1cb", bufs=2) as p1cb,
                tc.tile_pool(name="p1p", bufs=2, space="PSUM") as p1p,
            ):
                h_cur = hpool.tile([128, 96], BF16, tag="h")
                nc.vector.memset(h_cur[:], 0.0)
                c_cur = p1c.tile([128, 3, 32], F32, tag="c")
                nc.vector.memset(c_cur[:], 0.0)
                for m in range(3):
                    nc.sync.dma_start(cs[m].ap()[0], zero_t[:])

                h_fin = None
                for w in range(NCH2):
                    t0 = w * CH2
                    win = p1w.tile([128, 12, CH2, 32], BF16, tag="xwbwin")
                    nc.sync.dma_start(win[:], xwb2.ap()[w])
                    cbuf = p1cb.tile([128, 3, CH2, 32], BF16, tag="cbuf")
                    for j in range(CH2):
                        t = t0 + j
                        gp = p1p.tile([128, 12, 32], F32, tag="gates")
                        nc.tensor.matmul(gp[:], id_t[:], win[:, :, j, :],
                                         start=True, stop=False)
                        for s in range(12):
                            nc.tensor.matmul(
                                gp[:, s, :],
                                whhT_t[:, s * 128:(s + 1) * 128],
                                h_cur[:, (s % 3) * 32:(s % 3 + 1) * 32],
                                start=False, stop=(s == 11))
                        # tanh(g) first, then sigmoids (i,f), then o
                        tg = p1s.tile([128, 3, 32], F32, tag="tg")
                        nc.scalar.activation(tg[:], gp[:, 9:12, :], AF.Tanh)
                        sg = p1s.tile([128, 6, 32], F32, tag="sg")
                        nc.scalar.activation(sg[:], gp[:, 0:6, :], AF.Sigmoid)
                        so = p1s.tile([128, 3, 32], F32, tag="so")
                        nc.scalar.activation(so[:], gp[:, 6:9, :], AF.Sigmoid)
                        t1 = p1s.tile([128, 3, 32], F32, tag="t1")
                        nc.vector.tensor_mul(t1[:], sg[:, 0:3, :], tg[:])
                        t2 = p1s.tile([128, 3, 32], F32, tag="t2")
                        nc.gpsimd.tensor_mul(t2[:], sg[:, 3:6, :], c_cur[:])
                        c_new = p1c.tile([128, 3, 32], F32, tag="c")
                        nc.vector.tensor_add(c_new[:], t1[:], t2[:])
                        # bf16 copy of c for phase 2a (off critical path)
                        nc.gpsimd.tensor_copy(cbuf[:, :, j, :], c_new[:])
                        tc_t = p1s.tile([128, 3, 32], F32, tag="tc")
                        nc.scalar.activation(tc_t[:], c_new[:], AF.Tanh)
                        h_new = hpool.tile([128, 96], BF16, tag="h")
                        nc.vector.tensor_mul(
                            h_new[:].rearrange("p (m b) -> p m b", b=32),
                            so[:], tc_t[:])
                        c_cur = c_new
                        h_cur = h_new
                    for m in range(3):
                        nc.sync.dma_start(
                            cs[m].ap()[t0 + 1:t0 + CH2 + 1].transpose([1, 0, 2]),
                            cbuf[:, m, :, :])
                h_fin = h_cur

            tc.strict_bb_all_engine_barrier()

            # =================== PHASE 2a: time-parallel attention ===================
            with (
                tc.tile_pool(name="p2c", bufs=2) as p2c,
                tc.tile_pool(name="p2s", bufs=2) as p2s,
                tc.tile_pool(name="p2r", bufs=3) as p2r,
                tc.tile_pool(name="p2p1", bufs=2, space="PSUM") as p2p1,
                tc.tile_pool(name="p2pe", bufs=2, space="PSUM") as p2pe,
                tc.tile_pool(name="p2po", bufs=2, space="PSUM") as p2po,
                tc.tile_pool(name="p2ps", bufs=1, space="PSUM") as p2ps,
            ):
                for k in range(NCH2):
                    t0 = k * CH2
                    cw = [p2c.tile([128, CH2 + 1, 32], BF16, tag=f"cw{m}", name=f"cw{m}")
                          for m in range(3)]
                    for m in range(3):
                        nc.sync.dma_start(
                            cw[m][:], cs[m].ap()[t0:t0 + CH2 + 1].transpose([1, 0, 2]))

                    def rhs_k(kc):
                        if kc < 3:
                            return cw[kc][:, 0:CH2, :]
                        return cw[kc - 3][:, 1:CH2 + 1, :]

                    # att1 layer 1 + relu
                    y1p = p2p1.tile([128, 2, NB2], F32, tag="stage1")
                    for mc in range(2):
                        for kc in range(6):
                            nc.tensor.matmul(
                                y1p[:, mc, :], a1w1_t[:, kc, mc * 128:(mc + 1) * 128],
                                rhs_k(kc), start=(kc == 0), stop=(kc == 5))
                    y1 = p2s.tile([128, 2, NB2], BF16, tag="y1")
                    for mc in range(2):
                        nc.scalar.activation(y1[:, mc, :], y1p[:, mc, :], AF.Relu,
                                             bias=a1b1_t[:, mc:mc + 1])
                    # att1 layer 2 + exp (eager per-block to limit PSUM use)
                    et = p2s.tile([128, 6, NB2], BF16, tag="et")
                    for mc6 in range(6):
                        ep = p2pe.tile([128, NB2], F32, tag="logits")
                        for kc in range(2):
                            nc.tensor.matmul(
                                ep[:], a1w2_t[:, kc, mc6 * 128:(mc6 + 1) * 128],
                                y1[:, kc, :], start=(kc == 0), stop=(kc == 1))
                        nc.scalar.activation(et[:, mc6, :], ep[:], AF.Exp,
                                             bias=a1b2_t[:, mc6:mc6 + 1])
                    # S = sum over features of E; Sinv broadcast
                    sp = p2ps.tile([1, NB2], F32, tag="srow")
                    for q in range(6):
                        nc.tensor.matmul(sp[:], ones128_t[:], et[:, q, :],
                                         start=(q == 0), stop=(q == 5))
                    srec = p2r.tile([1, NB2], F32, tag="srec")
                    nc.vector.reciprocal(srec[:], sp[:])
                    srb = p2r.tile([1, NB2], BF16, tag="srb")
                    nc.gpsimd.tensor_copy(srb[:], srec[:])
                    sbp = p2ps.tile([128, NB2], F32, tag="sbp")
                    nc.tensor.matmul(sbp[:], ones1x128_t[:], srb[:],
                                     start=True, stop=True)
                    sbb = p2s.tile([128, NB2], BF16, tag="sbb")
                    nc.scalar.copy(sbb[:], sbp[:])

                    # U = E * cStar, A = U * Sinv (normalized attended)
                    ut = p2s.tile([128, 6, NB2], BF16, tag="ut")
                    for q in range(6):
                        nc.vector.tensor_mul(ut[:, q, :], et[:, q, :], rhs_k(q))
                    at = p2s.tile([128, 6, NB2], BF16, tag="at")
                    nc.vector.tensor_mul(
                        at[:], ut[:], sbb[:].unsqueeze(1).broadcast_to([128, 6, NB2]))

                    # att2 layer 1 + relu (+b1 via act bias)
                    zp = p2p1.tile([128, 2, NB2], F32, tag="stage1")
                    for mc in range(2):
                        for kc in range(6):
                            nc.tensor.matmul(
                                zp[:, mc, :], a2w1_t[:, kc, mc * 128:(mc + 1) * 128],
                                at[:, kc, :], start=(kc == 0), stop=(kc == 5))
                    z = p2s.tile([128, 2, NB2], BF16, tag="z")
                    for mc in range(2):
                        nc.scalar.activation(z[:, mc, :], zp[:, mc, :], AF.Relu,
                                             bias=a2b1f_t[:, mc:mc + 1])
                    # att2 layer 2 + tanh (+b2) -> cHat fully computed
                    ap2 = p2po.tile([128, 2, NB2], F32, tag="out")
                    for mc in range(2):
                        for kc in range(2):
                            nc.tensor.matmul(
                                ap2[:, mc, :], a2w2_t[:, kc, mc * 128:(mc + 1) * 128],
                                z[:, kc, :], start=(kc == 0), stop=(kc == 1))
                    chs = p2s.tile([128, 2, NB2], BF16, tag="chs")
                    for mc in range(2):
                        nc.scalar.activation(chs[:, mc, :], ap2[:, mc, :], AF.Tanh,
                                             bias=a2b2f_t[:, mc:mc + 1])
                    nc.sync.dma_start(chat_d.ap()[k], chs[:].rearrange("p a b -> p (a b)"))

                    # g1 / g2 attended-part + b1 (stored pre-relu, mem part added in ph3)
                    for gi, (gw, gbf) in enumerate(
                            ((g1a_t, g1b1f_t), (g2a_t, g2b1f_t))):
                        gp2 = p2po.tile([128, 2, NB2], F32, tag="out")
                        for mc in range(2):
                            for kc in range(6):
                                nc.tensor.matmul(
                                    gp2[:, mc, :], gw[:, kc, mc * 128:(mc + 1) * 128],
                                    at[:, kc, :], start=(kc == 0), stop=(kc == 5))
                        gs = p2s.tile([128, 2, NB2], BF16, tag=f"g{gi}s")
                        for mc in range(2):
                            if gi == 0:
                                nc.vector.tensor_scalar_add(gs[:, mc, :], gp2[:, mc, :],
                                                            gbf[:, mc:mc + 1])
                            else:
                                nc.scalar.activation(gs[:, mc, :], gp2[:, mc, :],
                                                     AF.Identity,
                                                     bias=gbf[:, mc:mc + 1])
                        nc.sync.dma_start(
                            gp_d.ap()[k, :, gi * 2 * NB2:(gi + 1) * 2 * NB2],
                            gs[:].rearrange("p a b -> p (a b)"))

            tc.strict_bb_all_engine_barrier()

            # =================== PHASE 3: memory recurrence ===================
            with (
                tc.tile_pool(name="p3w", bufs=2) as p3w,
                tc.tile_pool(name="p3s", bufs=3) as p3s,
                tc.tile_pool(name="p3m", bufs=2) as p3m,
                tc.tile_pool(name="p3p", bufs=2, space="PSUM") as p3p,
                tc.tile_pool(name="p3pb", bufs=2, space="PSUM") as p3pb,
            ):
                mem_cur = p3m.tile([128, 2, 32], BF16, tag="mem")
                nc.vector.memset(mem_cur[:], 0.0)
                for k in range(NCH2):
                    gw_t = p3w.tile([128, 4, CH2, 32], BF16, tag="gw")
                    nc.sync.dma_start(
                        gw_t[:], gp_d.ap()[k].rearrange("p (r t b) -> p r t b", r=4, b=32))
                    chw = p3w.tile([128, 2, CH2, 32], BF16, tag="chw")
                    nc.sync.dma_start(
                        chw[:], chat_d.ap()[k].rearrange("p (a t b) -> p a t b", a=2, b=32))

                    for j in range(CH2):
                        # L1: inject att-part (+b1 already), add mem-part matmuls
                        pg = p3p.tile([128, 4, 32], F32, tag="gmm")
                        nc.tensor.matmul(
                            pg[:], id_t[:], gw_t[:, :, j, :],
                            start=True, stop=False)
                        for r, gwt in enumerate((g1b_t, g1b_t, g2b_t, g2b_t)):
                            mc = r % 2
                            for kc in range(2):
                                nc.tensor.matmul(
                                    pg[:, r, :], gwt[:, kc, mc * 128:(mc + 1) * 128],
                                    mem_cur[:, kc, :], start=False,
                                    stop=(r == 3 and kc == 1))
                        hh = p3s.tile([128, 4, 32], BF16, tag="hh")
                        nc.scalar.activation(hh[:], pg[:], AF.Relu)
                        # L2 + b2 fold (K=4 selector inject; runs before hh ready)
                        qg = p3pb.tile([128, 4, 32], F32, tag="qmm")
                        nc.tensor.matmul(qg[:], b2f4_t[:], e4sel_t[:],
                                         start=True, stop=False)
                        for r, gwt in enumerate((g1w2_t, g1w2_t, g2w2_t, g2w2_t)):
                            mc = r % 2
                            goff = 0 if r < 2 else 2
                            for kc in range(2):
                                nc.tensor.matmul(
                                    qg[:, r, :], gwt[:, kc, mc * 128:(mc + 1) * 128],
                                    hh[:, goff + kc, :], start=False,
                                    stop=(r == 3 and kc == 1))
                        gam = p3s.tile([128, 4, 32], BF16, tag="gam")
                        nc.scalar.activation(gam[:], qg[:], AF.Sigmoid)
                        # mem = gam1*mem + gam2*cHat
                        m1 = p3s.tile([128, 2, 32], BF16, tag="m1")
                        nc.vector.tensor_mul(m1[:], gam[:, 0:2, :], mem_cur[:])
                        m2 = p3s.tile([128, 2, 32], BF16, tag="m2")
                        nc.gpsimd.tensor_mul(m2[:], gam[:, 2:4, :], chw[:, :, j, :])
                        mem_new = p3m.tile([128, 2, 32], BF16, tag="mem")
                        nc.vector.tensor_add(mem_new[:], m1[:], m2[:])
                        mem_cur = mem_new

                # =================== PHASE 4: output MLP ===================
                with tc.tile_pool(name="p4p", bufs=1, space="PSUM") as p4p:
                    o1p = p4p.tile([128, 2, 32], F32, tag="o1")
                    rhs5 = [h_fin[:, 0:32], h_fin[:, 32:64], h_fin[:, 64:96],
                            mem_cur[:, 0, :], mem_cur[:, 1, :]]
                    for mc in range(2):
                        for kc in range(5):
                            nc.tensor.matmul(
                                o1p[:, mc, :], ow1_t[:, kc, mc * 128:(mc + 1) * 128],
                                rhs5[kc], start=(kc == 0), stop=(kc == 4))
                    o1s = p3s.tile([128, 2, 32], BF16, tag="o1s")
                    for mc in range(2):
                        nc.scalar.activation(o1s[:, mc, :], o1p[:, mc, :], AF.Relu,
                                             bias=ob1_t[:, mc:mc + 1])
                    o2p = p4p.tile([1, 32], F32, tag="o2")
                    for kc in range(2):
                        nc.tensor.matmul(o2p[:], ow2_t[:, kc, :], o1s[:, kc, :],
                                         start=(kc == 0), stop=(kc == 1))
                    o2s = p3s.tile([1, 32], F32, tag="o2s")
                    nc.scalar.activation(o2s[:], o2p[:], AF.Identity, bias=ob2_t[:])
                    nc.sync.dma_start(out_d.ap().rearrange("b one -> (one) (b)"), o2s[:])

    nc.compile()
    return nc


# ---------------------------------------------------------------------------
# host-side packing
# ---------------------------------------------------------------------------

def pack_shared(inp):
    """Pack weight tensors (identical across cores)."""
    f = np.float32
    d = {}
    wih = {0: inp["Wih_l"], 1: inp["Wih_a"], 2: inp["Wih_v"]}
    whh = {0: inp["Whh_l"], 1: inp["Whh_a"], 2: inp["Whh_v"]}
    bb = {m: (inp[f"bih_{k}"] + inp[f"bhh_{k}"]).astype(f)
          for m, k in ((0, "l"), (1, "a"), (2, "v"))}
    foff = {0: 0, 1: D_L, 2: D_L + D_A}
    din = {0: D_L, 1: D_A, 2: D_V}

    waug = np.zeros((512, 1536), f)
    whhT = np.zeros((128, 1536), f)
    for gq in range(4):
        tg = TORCH_G[gq]
        for m in range(3):
            s = gq * 3 + m
            wblk = wih[m][tg * 128:(tg + 1) * 128, :]          # [128, din]
            waug[foff[m]:foff[m] + din[m], s * 128:(s + 1) * 128] = wblk.T
            waug[DIN, s * 128:(s + 1) * 128] = bb[m][tg * 128:(tg + 1) * 128]
            whhT[:, s * 128:(s + 1) * 128] = whh[m][tg * 128:(tg + 1) * 128, :].T
    d["waug"] = waug.astype(NPBF)
    d["whhT"] = whhT.astype(NPBF)
    d["ident"] = np.eye(128, dtype=f).astype(NPBF)
    d["ones128"] = np.ones((128, 1), f).astype(NPBF)

    bf = lambda a: np.ascontiguousarray(np.asarray(a, f)).astype(NPBF)
    fm2 = lambda b: np.ascontiguousarray(np.asarray(b, f).reshape(2, 128).T)  # [128,2]

    d["a1w1"] = bf(np.asarray(inp["att1_W1"]).T)              # [768, 256]
    d["a1b1"] = fm2(inp["att1_b1"])
    d["a1w2"] = bf(np.asarray(inp["att1_W2"]).T)              # [256, 768]
    d["a1b2"] = np.ascontiguousarray(np.asarray(inp["att1_b2"], f).reshape(6, 128).T)
    d["a2w1"] = bf(np.asarray(inp["att2_W1"]).T)
    d["a2b1f"] = fm2(inp["att2_b1"])
    d["a2w2"] = bf(np.asarray(inp["att2_W2"]).T)              # [256, 256]
    d["a2b2f"] = fm2(inp["att2_b2"])
    d["g1a"] = bf(np.asarray(inp["g1_W1"])[:, :768].T)
    d["g2a"] = bf(np.asarray(inp["g2_W1"])[:, :768].T)
    d["g1b"] = bf(np.asarray(inp["g1_W1"])[:, 768:].T)
    d["g2b"] = bf(np.asarray(inp["g2_W1"])[:, 768:].T)
    d["g1b1f"] = fm2(inp["g1_b1"])
    d["g2b1f"] = fm2(inp["g2_b1"])
    d["g1w2"] = bf(np.asarray(inp["g1_W2"]).T)
    d["g2w2"] = bf(np.asarray(inp["g2_W2"]).T)
    d["b2f4"] = bf(np.concatenate([np.asarray(inp["g1_b2"]),
                                   np.asarray(inp["g2_b2"])]).reshape(4, 128))
    d["e4sel"] = bf(np.kron(np.eye(4, dtype=f), np.ones((1, 32), f)))
    d["ow1"] = bf(np.asarray(inp["out_W1"]).T)                # [640, 256]
    d["ob1"] = fm2(inp["out_b1"])
    d["ow2"] = bf(np.asarray(inp["out_W2"]).T)                # [256, 1]
    d["ob2"] = np.asarray(inp["out_b2"], f).reshape(1, 1).copy()
    return d


def pack_x(x, core, Tp):
    """x: [Tp, 256, 409] -> xT [410, Tp*B] bf16 for one core."""
    xc = np.asarray(x[:, core * B:(core + 1) * B, :], np.float32)   # [Tp, B, 409]
    xt = xc.transpose(2, 0, 1).reshape(DIN, Tp * B)
    return np.concatenate([xt, np.ones((1, Tp * B), np.float32)], 0).astype(NPBF)


_CACHE = {}


def _get_program(Tp):
    if Tp not in _CACHE:
        _CACHE[Tp] = build_program(Tp)
    return _CACHE[Tp]


def kernel(**inputs):
    x = np.asarray(inputs["x"])
    Tp = x.shape[0]
    nc = _get_program(Tp)
    shared = pack_shared({k: np.asarray(v) for k, v in inputs.items()})
    in_maps = []
    for c in range(NCORES):
        m = dict(shared)
        m["xT"] = np.ascontiguousarray(pack_x(x, c, Tp))
        in_maps.append(m)
    res = run_bass_kernel_spmd(nc, in_maps, list(range(NCORES))).results
    out = np.concatenate([r["out"] for r in res], axis=0)
    return out.astype(np.float32)


if __name__ == "__main__":
    import time
    t0 = time.time()
    nc = build_program(32)
    print("built in", time.time() - t0, "s")


# revision 18
# speedup vs baseline: 4.2775x; 1.6907x over previous
"""Trainium2 Bass kernel for nn_Contextual_MFN (Memory Fusion Network).

v3: fully fused chunk pipeline, all-bf16 matmuls, everything SBUF-resident.

Per 8-step chunk k, five pipeline stages run at different lags in one loop:
  ph0(i):   xWb = Wih_aug @ x_aug   -> win ring        (time-parallel)
  ph1(i-1): 3xLSTM recurrence       -> cbuf ring, h    (serial)
  a2a(i-2): att1 MLP, exp, U=E*cStar, S=sum E, 1/S     (time-parallel)
  b2a(i-3): A=U/S, att2 MLP->cHat, g-att-parts         (time-parallel)
  ph3(i-4): memory-gate recurrence  -> mem             (serial)
Emission interleaves serial-phase steps with parallel-phase matmul groups so
the in-order PE queue never stalls and the PE stays HAM-warm.

All activations feature-major: [features(partitions), batch(free)].
"""
import numpy as np
import ml_dtypes

import concourse.bass as bass
import concourse.tile as tile
from concourse import bacc, mybir
from concourse.bass_utils import run_bass_kernel_spmd

F32 = mybir.dt.float32
BF16 = mybir.dt.bfloat16
AF = mybir.ActivationFunctionType
NPBF = ml_dtypes.bfloat16

T_FULL = 512
NBATCH = 256
NCORES = 8
B = NBATCH // NCORES          # 32 batch rows per core
D_L, D_A, D_V = 300, 74, 35
DIN = D_L + D_A + D_V         # 409
DAUG = DIN + 1                # 410 (ones row for bias)
DH = 128
MEM = 256
CH = 8                        # chunk length (steps) for all stages
NB2 = CH * B                  # 256

TORCH_G = (0, 1, 3, 2)        # slot g' -> torch gate row block; s = g'*3 + m


def _nonzero_kcs(s):
    m = s % 3
    if m == 0:
        return [0, 1, 2, 3]
    return [2, 3]


def build_program(Tp=T_FULL):
    assert Tp % CH == 0
    NCH = Tp // CH
    nc = bacc.Bacc("TRN2", target_bir_lowering=False, debug=False)

    # ---------------- external inputs ----------------
    xT = nc.dram_tensor("xT", [DAUG, Tp * B], BF16, kind="ExternalInput")
    waug = nc.dram_tensor("waug", [512, 1536], BF16, kind="ExternalInput")
    whhT = nc.dram_tensor("whhT", [128, 1536], BF16, kind="ExternalInput")
    ident = nc.dram_tensor("ident", [128, 128], BF16, kind="ExternalInput")
    ones128 = nc.dram_tensor("ones128", [128, 1], BF16, kind="ExternalInput")

    a1w1 = nc.dram_tensor("a1w1", [768, 256], BF16, kind="ExternalInput")
    a1b1 = nc.dram_tensor("a1b1", [128, 2], F32, kind="ExternalInput")
    a1w2 = nc.dram_tensor("a1w2", [256, 768], BF16, kind="ExternalInput")
    a1b2 = nc.dram_tensor("a1b2", [128, 6], F32, kind="ExternalInput")
    a2w1 = nc.dram_tensor("a2w1", [768, 256], BF16, kind="ExternalInput")
    a2b1f = nc.dram_tensor("a2b1f", [128, 2], F32, kind="ExternalInput")
    a2w2 = nc.dram_tensor("a2w2", [256, 256], BF16, kind="ExternalInput")
    a2b2f = nc.dram_tensor("a2b2f", [128, 2], F32, kind="ExternalInput")
    g1a = nc.dram_tensor("g1a", [768, 256], BF16, kind="ExternalInput")
    g2a = nc.dram_tensor("g2a", [768, 256], BF16, kind="ExternalInput")
    g1b = nc.dram_tensor("g1b", [256, 256], BF16, kind="ExternalInput")
    g2b = nc.dram_tensor("g2b", [256, 256], BF16, kind="ExternalInput")
    g1b1f = nc.dram_tensor("g1b1f", [128, 2], F32, kind="ExternalInput")
    g2b1f = nc.dram_tensor("g2b1f", [128, 2], F32, kind="ExternalInput")
    g1w2 = nc.dram_tensor("g1w2", [256, 256], BF16, kind="ExternalInput")
    g2w2 = nc.dram_tensor("g2w2", [256, 256], BF16, kind="ExternalInput")
    b2f4 = nc.dram_tensor("b2f4", [4, 128], BF16, kind="ExternalInput")
    e4sel = nc.dram_tensor("e4sel", [4, 128], BF16, kind="ExternalInput")
    ow1 = nc.dram_tensor("ow1", [640, 256], BF16, kind="ExternalInput")
    ob1 = nc.dram_tensor("ob1", [128, 2], F32, kind="ExternalInput")
    ow2 = nc.dram_tensor("ow2", [256, 1], BF16, kind="ExternalInput")
    ob2 = nc.dram_tensor("ob2", [1, 1], F32, kind="ExternalInput")

    out_d = nc.dram_tensor("out", [B, 1], F32, kind="ExternalOutput")

    import contextlib
    with tile.TileContext(nc) as tc:
        ctx = contextlib.ExitStack()
        with ctx:
            wpool = ctx.enter_context(tc.tile_pool(name="weights", bufs=1))
            hpool = ctx.enter_context(tc.tile_pool(name="hstate", bufs=2))
            rpool = ctx.enter_context(tc.tile_pool(name="rings", bufs=2))
            spool = ctx.enter_context(tc.tile_pool(name="scratch", bufs=2))
            mpool = ctx.enter_context(tc.tile_pool(name="memstate", bufs=2))
            # PSUM: exactly 8 banks
            pacc = ctx.enter_context(tc.tile_pool(name="pacc", bufs=2, space="PSUM"))
            pst1 = ctx.enter_context(tc.tile_pool(name="pst1", bufs=1, space="PSUM"))
            pout = ctx.enter_context(tc.tile_pool(name="pout", bufs=1, space="PSUM"))
            psf = ctx.enter_context(tc.tile_pool(name="psf", bufs=1, space="PSUM"))
            p1g = ctx.enter_context(tc.tile_pool(name="p1g", bufs=1, space="PSUM"))
            p3g = ctx.enter_context(tc.tile_pool(name="p3g", bufs=2, space="PSUM"))

            # ---- resident weights / constants ----
            wihT_t = wpool.tile([128, 4, 1536], BF16)
            nc.sync.dma_start(
                wihT_t[:], waug.ap().rearrange("(kc p) c -> p kc c", p=128))
            whhT_t = wpool.tile([128, 1536], BF16)
            nc.sync.dma_start(whhT_t[:], whhT.ap())
            id_t = wpool.tile([128, 128], BF16)
            nc.sync.dma_start(id_t[:], ident.ap())
            ones128_t = wpool.tile([128, 1], BF16)
            nc.sync.dma_start(ones128_t[:], ones128.ap())
            ones1x128_t = wpool.tile([1, 128], BF16)
            nc.vector.memset(ones1x128_t[:], 1.0)

            a1w1_t = wpool.tile([128, 6, 256], BF16)
            nc.sync.dma_start(a1w1_t[:], a1w1.ap().rearrange("(kc p) c -> p kc c", p=128))
            a1b1_t = wpool.tile([128, 2], F32)
            nc.sync.dma_start(a1b1_t[:], a1b1.ap())
            a1w2_t = wpool.tile([128, 2, 768], BF16)
            nc.sync.dma_start(a1w2_t[:], a1w2.ap().rearrange("(kc p) c -> p kc c", p=128))
            a1b2_t = wpool.tile([128, 6], F32)
            nc.sync.dma_start(a1b2_t[:], a1b2.ap())
            a2w1_t = wpool.tile([128, 6, 256], BF16)
            nc.sync.dma_start(a2w1_t[:], a2w1.ap().rearrange("(kc p) c -> p kc c", p=128))
            a2b1f_t = wpool.tile([128, 2], F32)
            nc.sync.dma_start(a2b1f_t[:], a2b1f.ap())
            a2w2_t = wpool.tile([128, 2, 256], BF16)
            nc.sync.dma_start(a2w2_t[:], a2w2.ap().rearrange("(kc p) c -> p kc c", p=128))
            a2b2f_t = wpool.tile([128, 2], F32)
            nc.sync.dma_start(a2b2f_t[:], a2b2f.ap())
            g1a_t = wpool.tile([128, 6, 256], BF16)
            nc.sync.dma_start(g1a_t[:], g1a.ap().rearrange("(kc p) c -> p kc c", p=128))
            g2a_t = wpool.tile([128, 6, 256], BF16)
            nc.sync.dma_start(g2a_t[:], g2a.ap().rearrange("(kc p) c -> p kc c", p=128))
            g1b1f_t = wpool.tile([128, 2], F32)
            nc.sync.dma_start(g1b1f_t[:], g1b1f.ap())
            g2b1f_t = wpool.tile([128, 2], F32)
            nc.sync.dma_start(g2b1f_t[:], g2b1f.ap())
            g1b_t = wpool.tile([128, 2, 256], BF16)
            nc.sync.dma_start(g1b_t[:], g1b.ap().rearrange("(kc p) c -> p kc c", p=128))
            g2b_t = wpool.tile([128, 2, 256], BF16)
            nc.sync.dma_start(g2b_t[:], g2b.ap().rearrange("(kc p) c -> p kc c", p=128))
            g1w2_t = wpool.tile([128, 2, 256], BF16)
            nc.sync.dma_start(g1w2_t[:], g1w2.ap().rearrange("(kc p) c -> p kc c", p=128))
            g2w2_t = wpool.tile([128, 2, 256], BF16)
            nc.sync.dma_start(g2w2_t[:], g2w2.ap().rearrange("(kc p) c -> p kc c", p=128))
            b2f4_t = wpool.tile([4, 128], BF16)
            nc.sync.dma_start(b2f4_t[:], b2f4.ap())
            e4sel_t = wpool.tile([4, 128], BF16)
            nc.sync.dma_start(e4sel_t[:], e4sel.ap())
            ow1_t = wpool.tile([128, 5, 256], BF16)
            nc.sync.dma_start(ow1_t[:], ow1.ap().rearrange("(kc p) c -> p kc c", p=128))
            ob1_t = wpool.tile([128, 2], F32)
            nc.sync.dma_start(ob1_t[:], ob1.ap())
            ow2_t = wpool.tile([128, 2, 1], BF16)
            nc.sync.dma_start(ow2_t[:], ow2.ap().rearrange("(kc p) c -> p kc c", p=128))
            ob2_t = wpool.tile([1, 1], F32)
            nc.sync.dma_start(ob2_t[:], ob2.ap())

            # ---- persistent state ----
            st = {}
            st["h"] = hpool.tile([128, 96], BF16, tag="h", name="h0")
            nc.vector.memset(st["h"][:], 0.0)
            st["c"] = hpool.tile([128, 3, 32], F32, tag="c", name="c0")
            nc.vector.memset(st["c"][:], 0.0)
            st["mem"] = mpool.tile([128, 2, 32], BF16, tag="mem", name="mem0")
            nc.vector.memset(st["mem"][:], 0.0)

            # ring tile dicts keyed by chunk
            WIN, CB, ET, UT, SRB, CHS, GS = {}, {}, {}, {}, {}, {}, {}

            # ---------------- emission units ----------------
            def ph0_units(k):
                t0 = k * CH
                units = []
                xt = spool.tile([128, 4, NB2], BF16, tag="xt", name=f"xt{k}")
                win = rpool.tile([128, 12, CH, 32], BF16, tag="win", name=f"win{k}")
                WIN[k] = win

                def load():
                    for kc in range(4):
                        rows = 128 if kc < 3 else DAUG - 384
                        nc.sync.dma_start(
                            xt[0:rows, kc, :],
                            xT.ap()[kc * 128:kc * 128 + rows, t0 * B:(t0 + CH) * B])
                units.append(load)

                def slot(s):
                    def f():
                        pt = pacc.tile([128, NB2], F32, tag="acc", name=f"p0a{k}_{s}")
                        kcs = _nonzero_kcs(s)
                        for i, kc in enumerate(kcs):
                            rows = 128 if kc < 3 else DAUG - 384
                            nc.tensor.matmul(
                                pt[:], wihT_t[0:rows, kc, s * 128:(s + 1) * 128],
                                xt[0:rows, kc, :],
                                start=(i == 0), stop=(i == len(kcs) - 1))
                        dst = win[:, s, :, :]
                        if s % 2 == 0:
                            nc.vector.tensor_copy(dst, pt[:].rearrange("p (t b) -> p t b", b=B))
                        else:
                            nc.scalar.copy(dst, pt[:].rearrange("p (t b) -> p t b", b=B))
                    return f
                units += [slot(s) for s in range(12)]
                return units

            def ph1_step(k, j):
                win = WIN[k]

                def f():
                    if j == 0:
                        cb = rpool.tile([128, 3, CH + 1, 32], BF16, tag="cb", name=f"cb{k}")
                        CB[k] = cb
                        nc.gpsimd.tensor_copy(cb[:, :, 0, :], st["c"][:])
                    cb = CB[k]
                    gp = p1g.tile([128, 12, 32], F32, tag="gates", name=f"g1p{k}_{j}")
                    nc.tensor.matmul(gp[:], id_t[:], win[:, :, j, :],
                                     start=True, stop=False)
                    for s in range(12):
                        nc.tensor.matmul(
                            gp[:, s, :], whhT_t[:, s * 128:(s + 1) * 128],
                            st["h"][:, (s % 3) * 32:(s % 3 + 1) * 32],
                            start=False, stop=(s == 11))
                    tg = spool.tile([128, 3, 32], F32, tag="tg", name=f"tg{k}_{j}")
                    nc.scalar.activation(tg[:], gp[:, 9:12, :], AF.Tanh)
                    sg = spool.tile([128, 9, 32], F32, tag="sg", name=f"sg{k}_{j}")
                    nc.scalar.activation(sg[:], gp[:, 0:9, :], AF.Sigmoid)
                    t1 = spool.tile([128, 3, 32], F32, tag="t1", name=f"t1{k}_{j}")
                    nc.gpsimd.tensor_mul(t1[:], sg[:, 0:3, :], tg[:])
                    t2 = spool.tile([128, 3, 32], F32, tag="t2", name=f"t2{k}_{j}")
                    nc.vector.tensor_mul(t2[:], sg[:, 3:6, :], st["c"][:])
                    c_new = hpool.tile([128, 3, 32], F32, tag="c", name=f"c{k}_{j}")
                    nc.vector.tensor_add(c_new[:], t1[:], t2[:])
                    nc.gpsimd.tensor_copy(cb[:, :, j + 1, :], c_new[:])
                    tc_t = spool.tile([128, 3, 32], F32, tag="tc", name=f"tc{k}_{j}")
                    nc.scalar.activation(tc_t[:], c_new[:], AF.Tanh)
                    h_new = hpool.tile([128, 96], BF16, tag="h", name=f"h{k}_{j}")
                    nc.vector.tensor_mul(
                        h_new[:].rearrange("p (m b) -> p m b", b=32),
                        sg[:, 6:9, :], tc_t[:])
                    st["c"] = c_new
                    st["h"] = h_new
                return f

            def alpha_units(k):
                units = []
                cb = CB[k]

                def rhs_k(kc):
                    if kc < 3:
                        return cb[:, kc, 0:CH, :]
                    return cb[:, kc - 3, 1:CH + 1, :]

                y1p = pst1.tile([128, 2, NB2], F32, tag="stage1", name=f"y1p{k}")
                y1 = spool.tile([128, 2, NB2], BF16, tag="y1", name=f"y1{k}")

                def att1_l1(mc):
                    def f():
                        for kc in range(6):
                            nc.tensor.matmul(
                                y1p[:, mc, :], a1w1_t[:, kc, mc * 128:(mc + 1) * 128],
                                rhs_k(kc), start=(kc == 0), stop=(kc == 5))
                        nc.scalar.activation(y1[:, mc, :], y1p[:, mc, :], AF.Relu,
                                             bias=a1b1_t[:, mc:mc + 1])
                    return f
                units += [att1_l1(0), att1_l1(1)]

                et = rpool.tile([128, 6, NB2], BF16, tag="et", name=f"et{k}")
                ET[k] = et

                def logit(mc6):
                    def f():
                        ep = pacc.tile([128, NB2], F32, tag="acc", name=f"ep{k}_{mc6}")
                        for kc in range(2):
                            nc.tensor.matmul(
                                ep[:], a1w2_t[:, kc, mc6 * 128:(mc6 + 1) * 128],
                                y1[:, kc, :], start=(kc == 0), stop=(kc == 1))
                        nc.scalar.activation(et[:, mc6, :], ep[:], AF.Exp,
                                             bias=a1b2_t[:, mc6:mc6 + 1])
                    return f
                units += [logit(m6) for m6 in range(6)]

                ut = rpool.tile([128, 6, NB2], BF16, tag="ut", name=f"ut{k}")
                UT[k] = ut

                def umul(q):
                    def f():
                        nc.vector.tensor_mul(ut[:, q, :], et[:, q, :], rhs_k(q))
                    return f
                units += [umul(q) for q in range(6)]

                def ssum():
                    sp = psf.tile([1, NB2], F32, tag="srow", name=f"sp{k}")
                    for q in range(6):
                        nc.tensor.matmul(sp[:], ones128_t[:], et[:, q, :],
                                         start=(q == 0), stop=(q == 5))
                    srec = spool.tile([1, NB2], F32, tag="srec", name=f"sr{k}")
                    nc.vector.reciprocal(srec[:], sp[:])
                    srb = rpool.tile([1, NB2], BF16, tag="srb", name=f"srb{k}")
                    SRB[k] = srb
                    nc.gpsimd.tensor_copy(srb[:], srec[:])
                units.append(ssum)
                return units

            def beta_units(k):
                units = []
                ut = UT[k]
                sbb = spool.tile([128, NB2], BF16, tag="sbb", name=f"sbb{k}")
                at = rpool.tile([128, 6, NB2], BF16, tag="at", name=f"at{k}")

                def bpre():
                    sbp = pout.tile([128, 2, NB2], F32, tag="out", name=f"sbp{k}")
                    nc.tensor.matmul(sbp[:, 0, :], ones1x128_t[:], SRB[k][:],
                                     start=True, stop=True)
                    nc.scalar.copy(sbb[:], sbp[:, 0, :])
                    nc.vector.tensor_mul(
                        at[:], ut[:], sbb[:].unsqueeze(1).broadcast_to([128, 6, NB2]))
                units.append(bpre)

                zp = pst1.tile([128, 2, NB2], F32, tag="stage1", name=f"zp{k}")
                z = spool.tile([128, 2, NB2], BF16, tag="z", name=f"z{k}")

                def att2_l1(mc):
                    def f():
                        for kc in range(6):
                            nc.tensor.matmul(
                                zp[:, mc, :], a2w1_t[:, kc, mc * 128:(mc + 1) * 128],
                                at[:, kc, :], start=(kc == 0), stop=(kc == 5))
                        nc.scalar.activation(z[:, mc, :], zp[:, mc, :], AF.Relu,
                                             bias=a2b1f_t[:, mc:mc + 1])
                    return f
                units += [att2_l1(0), att2_l1(1)]

                chs = rpool.tile([128, 2, CH, 32], BF16, tag="chs", name=f"chs{k}")
                CHS[k] = chs

                def att2_l2():
                    ap2 = pout.tile([128, 2, NB2], F32, tag="out", name=f"ap2{k}")
                    for mc in range(2):
                        for kc in range(2):
                            nc.tensor.matmul(
                                ap2[:, mc, :], a2w2_t[:, kc, mc * 128:(mc + 1) * 128],
                                z[:, kc, :], start=(kc == 0), stop=(kc == 1))
                    for mc in range(2):
                        nc.scalar.activation(
                            chs[:, mc, :, :],
                            ap2[:, mc, :].rearrange("p (t b) -> p t b", b=B),
                            AF.Tanh, bias=a2b2f_t[:, mc:mc + 1])
                units.append(att2_l2)

                gs = rpool.tile([128, 4, CH, 32], BF16, tag="gs", name=f"gs{k}")
                GS[k] = gs

                def gpart(gi):
                    gw, gbf = ((g1a_t, g1b1f_t), (g2a_t, g2b1f_t))[gi]

                    def f():
                        gp2 = pout.tile([128, 2, NB2], F32, tag="out", name=f"gp2{k}_{gi}")
                        for mc in range(2):
                            for kc in range(6):
                                nc.tensor.matmul(
                                    gp2[:, mc, :], gw[:, kc, mc * 128:(mc + 1) * 128],
                                    at[:, kc, :], start=(kc == 0), stop=(kc == 5))
                        for mc in range(2):
                            dst = gs[:, gi * 2 + mc, :, :]
                            src = gp2[:, mc, :].rearrange("p (t b) -> p t b", b=B)
                            if gi == 0:
                                nc.vector.tensor_scalar_add(dst, src, gbf[:, mc:mc + 1])
                            else:
                                nc.scalar.activation(dst, src, AF.Identity,
                                                     bias=gbf[:, mc:mc + 1])
                    return f
                units += [gpart(0), gpart(1)]
                return units

            def ph3_step(k, j):
                def f():
                    gw_t, chw = GS[k], CHS[k]
                    pq = p3g.tile([128, 8, 32], F32, tag="g3", name=f"pq{k}_{j}")
                    pg = pq[:, 0:4, :]
                    qg = pq[:, 4:8, :]
                    # L2 b2 inject first (independent of this step's deps)
                    nc.tensor.matmul(qg, b2f4_t[:], e4sel_t[:], start=True, stop=False)
                    # L1: att-part inject + mem matmuls
                    nc.tensor.matmul(pg, id_t[:], gw_t[:, :, j, :],
                                     start=True, stop=False)
                    for r, gwt in enumerate((g1b_t, g1b_t, g2b_t, g2b_t)):
                        mc = r % 2
                        for kc in range(2):
                            nc.tensor.matmul(
                                pq[:, r, :], gwt[:, kc, mc * 128:(mc + 1) * 128],
                                st["mem"][:, kc, :], start=False,
                                stop=(r == 3 and kc == 1))
                    hh = spool.tile([128, 4, 32], BF16, tag="hh", name=f"hh{k}_{j}")
                    nc.scalar.activation(hh[:], pg, AF.Relu)
                    for r, gwt in enumerate((g1w2_t, g1w2_t, g2w2_t, g2w2_t)):
                        mc = r % 2
                        goff = 0 if r < 2 else 2
                        for kc in range(2):
                            nc.tensor.matmul(
                                pq[:, 4 + r, :], gwt[:, kc, mc * 128:(mc + 1) * 128],
                                hh[:, goff + kc, :], start=False,
                                stop=(r == 3 and kc == 1))
                    gam = spool.tile([128, 4, 32], BF16, tag="gam", name=f"gam{k}_{j}")
                    nc.scalar.activation(gam[:], qg, AF.Sigmoid)
                    m1 = spool.tile([128, 2, 32], BF16, tag="m1", name=f"m1{k}_{j}")
                    nc.gpsimd.tensor_mul(m1[:], gam[:, 0:2, :], st["mem"][:])
                    m2 = spool.tile([128, 2, 32], BF16, tag="m2", name=f"m2{k}_{j}")
                    nc.vector.tensor_mul(m2[:], gam[:, 2:4, :], chw[:, :, j, :])
                    mem_new = mpool.tile([128, 2, 32], BF16, tag="mem", name=f"mem{k}_{j}")
                    nc.vector.tensor_add(mem_new[:], m1[:], m2[:])
                    st["mem"] = mem_new
                return f

            # ---------------- fused pipeline loop ----------------
            NITER = NCH + 4
            for i in range(NITER):
                fillers = []
                if i < NCH:
                    fillers += ph0_units(i)
                if 0 <= i - 2 < NCH:
                    fillers += alpha_units(i - 2)
                if 0 <= i - 3 < NCH:
                    fillers += beta_units(i - 3)
                steps = []
                if 0 <= i - 1 < NCH:
                    steps.append([ph1_step(i - 1, j) for j in range(CH)])
                if 0 <= i - 4 < NCH:
                    steps.append([ph3_step(i - 4, j) for j in range(CH)])
                # round-robin: serial steps first, fillers spread between
                nf = len(fillers)
                fi = 0
                for j in range(CH):
                    for sl in steps:
                        sl[j]()
                    take = (nf * (j + 1)) // CH - (nf * j) // CH
                    for _ in range(take):
                        fillers[fi]()
                        fi += 1
                assert fi == nf

            # ---------------- PHASE 4: output MLP ----------------
            if True:
                h_fin = st["h"]
                mem_fin = st["mem"]
                o1p = p3g.tile([128, 2, 32], F32, tag="g3", name="o1p")
                rhs5 = [h_fin[:, 0:32], h_fin[:, 32:64], h_fin[:, 64:96],
                        mem_fin[:, 0, :], mem_fin[:, 1, :]]
                for mc in range(2):
                    for kc in range(5):
                        nc.tensor.matmul(
                            o1p[:, mc, :], ow1_t[:, kc, mc * 128:(mc + 1) * 128],
                            rhs5[kc], start=(kc == 0), stop=(kc == 4))
                o1s = spool.tile([128, 2, 32], BF16, tag="o1s")
                for mc in range(2):
                    nc.scalar.activation(o1s[:, mc, :], o1p[:, mc, :], AF.Relu,
                                         bias=ob1_t[:, mc:mc + 1])
                o2p = psf.tile([1, 32], F32, tag="srow", name="o2p")
                for kc in range(2):
                    nc.tensor.matmul(o2p[:], ow2_t[:, kc, :], o1s[:, kc, :],
                                     start=(kc == 0), stop=(kc == 1))
                o2s = spool.tile([1, 32], F32, tag="o2s")
                nc.scalar.activation(o2s[:], o2p[:], AF.Identity, bias=ob2_t[:])
                nc.sync.dma_start(out_d.ap().rearrange("b one -> (one) (b)"), o2s[:])

    nc.compile()
    return nc


# ---------------------------------------------------------------------------
# host-side packing
# ---------------------------------------------------------------------------

def pack_shared(inp):
    f = np.float32
    d = {}
    wih = {0: inp["Wih_l"], 1: inp["Wih_a"], 2: inp["Wih_v"]}
    whh = {0: inp["Whh_l"], 1: inp["Whh_a"], 2: inp["Whh_v"]}
    bb = {m: (inp[f"bih_{k}"] + inp[f"bhh_{k}"]).astype(f)
          for m, k in ((0, "l"), (1, "a"), (2, "v"))}
    foff = {0: 0, 1: D_L, 2: D_L + D_A}
    din = {0: D_L, 1: D_A, 2: D_V}

    waug = np.zeros((512, 1536), f)
    whhT = np.zeros((128, 1536), f)
    for gq in range(4):
        tg = TORCH_G[gq]
        for m in range(3):
            s = gq * 3 + m
            wblk = wih[m][tg * 128:(tg + 1) * 128, :]
            waug[foff[m]:foff[m] + din[m], s * 128:(s + 1) * 128] = wblk.T
            waug[DIN, s * 128:(s + 1) * 128] = bb[m][tg * 128:(tg + 1) * 128]
            whhT[:, s * 128:(s + 1) * 128] = whh[m][tg * 128:(tg + 1) * 128, :].T
    d["waug"] = waug.astype(NPBF)
    d["whhT"] = whhT.astype(NPBF)
    d["ident"] = np.eye(128, dtype=f).astype(NPBF)
    d["ones128"] = np.ones((128, 1), f).astype(NPBF)

    bf = lambda a: np.ascontiguousarray(np.asarray(a, f)).astype(NPBF)
    fm2 = lambda b: np.ascontiguousarray(np.asarray(b, f).reshape(2, 128).T)

    d["a1w1"] = bf(np.asarray(inp["att1_W1"]).T)
    d["a1b1"] = fm2(inp["att1_b1"])
    d["a1w2"] = bf(np.asarray(inp["att1_W2"]).T)
    d["a1b2"] = np.ascontiguousarray(np.asarray(inp["att1_b2"], f).reshape(6, 128).T)
    d["a2w1"] = bf(np.asarray(inp["att2_W1"]).T)
    d["a2b1f"] = fm2(inp["att2_b1"])
    d["a2w2"] = bf(np.asarray(inp["att2_W2"]).T)
    d["a2b2f"] = fm2(inp["att2_b2"])
    d["g1a"] = bf(np.asarray(inp["g1_W1"])[:, :768].T)
    d["g2a"] = bf(np.asarray(inp["g2_W1"])[:, :768].T)
    d["g1b"] = bf(np.asarray(inp["g1_W1"])[:, 768:].T)
    d["g2b"] = bf(np.asarray(inp["g2_W1"])[:, 768:].T)
    d["g1b1f"] = fm2(inp["g1_b1"])
    d["g2b1f"] = fm2(inp["g2_b1"])
    d["g1w2"] = bf(np.asarray(inp["g1_W2"]).T)
    d["g2w2"] = bf(np.asarray(inp["g2_W2"]).T)
    d["b2f4"] = bf(np.concatenate([np.asarray(inp["g1_b2"]),
                                   np.asarray(inp["g2_b2"])]).reshape(4, 128))
    d["e4sel"] = bf(np.kron(np.eye(4, dtype=f), np.ones((1, 32), f)))
    d["ow1"] = bf(np.asarray(inp["out_W1"]).T)
    d["ob1"] = fm2(inp["out_b1"])
    d["ow2"] = bf(np.asarray(inp["out_W2"]).T)
    d["ob2"] = np.asarray(inp["out_b2"], f).reshape(1, 1).copy()
    return d


def pack_x(x, core, Tp):
    xc = np.asarray(x[:, core * B:(core + 1) * B, :], np.float32)
    xt = xc.transpose(2, 0, 1).reshape(DIN, Tp * B)
    return np.concatenate([xt, np.ones((1, Tp * B), np.float32)], 0).astype(NPBF)


_CACHE = {}


def _get_program(Tp):
    if Tp not in _CACHE:
        _CACHE[Tp] = build_program(Tp)
    return _CACHE[Tp]


def kernel(**inputs):
    x = np.asarray(inputs["x"])
    Tp = x.shape[0]
    nc = _get_program(Tp)
    shared = pack_shared({k: np.asarray(v) for k, v in inputs.items()})
    in_maps = []
    for c in range(NCORES):
        m = dict(shared)
        m["xT"] = np.ascontiguousarray(pack_x(x, c, Tp))
        in_maps.append(m)
    res = run_bass_kernel_spmd(nc, in_maps, list(range(NCORES))).results
    out = np.concatenate([r["out"] for r in res], axis=0)
    return out.astype(np.float32)


if __name__ == "__main__":
    import time
    t0 = time.time()
    nc = build_program(64)
    print("built in", time.time() - t0, "s")


# revision 25
# speedup vs baseline: 4.3992x; 1.0284x over previous
"""Trainium2 Bass kernel for nn_Contextual_MFN (Memory Fusion Network).

v3: fully fused chunk pipeline, all-bf16 matmuls, everything SBUF-resident.

Per 8-step chunk k, five pipeline stages run at different lags in one loop:
  ph0(i):   xWb = Wih_aug @ x_aug   -> win ring        (time-parallel)
  ph1(i-1): 3xLSTM recurrence       -> cbuf ring, h    (serial)
  a2a(i-2): att1 MLP, exp, U=E*cStar, S=sum E, 1/S     (time-parallel)
  b2a(i-3): A=U/S, att2 MLP->cHat, g-att-parts         (time-parallel)
  ph3(i-4): memory-gate recurrence  -> mem             (serial)
Emission interleaves serial-phase steps with parallel-phase matmul groups so
the in-order PE queue never stalls and the PE stays HAM-warm.

All activations feature-major: [features(partitions), batch(free)].
"""
import numpy as np
import ml_dtypes

import concourse.bass as bass
import concourse.tile as tile
from concourse import bacc, mybir
from concourse.bass_utils import run_bass_kernel_spmd

F32 = mybir.dt.float32
BF16 = mybir.dt.bfloat16
AF = mybir.ActivationFunctionType
NPBF = ml_dtypes.bfloat16

T_FULL = 512
NBATCH = 256
NCORES = 8
B = NBATCH // NCORES          # 32 batch rows per core
D_L, D_A, D_V = 300, 74, 35
DIN = D_L + D_A + D_V         # 409
DAUG = DIN + 1                # 410 (ones row for bias)
DH = 128
MEM = 256
CH = 8                        # chunk length (steps) for all stages
NB2 = CH * B                  # 256

TORCH_G = (0, 1, 3, 2)        # slot g' -> torch gate row block; s = g'*3 + m


def _nonzero_kcs(s):
    m = s % 3
    if m == 0:
        return [0, 1, 2, 3]
    return [2, 3]


def build_program(Tp=T_FULL):
    assert Tp % CH == 0
    NCH = Tp // CH
    nc = bacc.Bacc("TRN2", target_bir_lowering=False, debug=False)

    # ---------------- external inputs ----------------
    xT = nc.dram_tensor("xT", [DAUG, Tp * B], BF16, kind="ExternalInput")
    waug = nc.dram_tensor("waug", [512, 1536], BF16, kind="ExternalInput")
    whhT = nc.dram_tensor("whhT", [128, 1536], BF16, kind="ExternalInput")
    ident = nc.dram_tensor("ident", [128, 128], BF16, kind="ExternalInput")
    ones128 = nc.dram_tensor("ones128", [128, 1], BF16, kind="ExternalInput")

    a1w1 = nc.dram_tensor("a1w1", [768, 256], BF16, kind="ExternalInput")
    a1b1 = nc.dram_tensor("a1b1", [128, 2], F32, kind="ExternalInput")
    a1w2 = nc.dram_tensor("a1w2", [256, 768], BF16, kind="ExternalInput")
    a1b2 = nc.dram_tensor("a1b2", [128, 6], F32, kind="ExternalInput")
    a2w1 = nc.dram_tensor("a2w1", [768, 256], BF16, kind="ExternalInput")
    a2b1f = nc.dram_tensor("a2b1f", [128, 2], F32, kind="ExternalInput")
    a2w2 = nc.dram_tensor("a2w2", [256, 256], BF16, kind="ExternalInput")
    a2b2f = nc.dram_tensor("a2b2f", [128, 2], F32, kind="ExternalInput")
    g1a = nc.dram_tensor("g1a", [768, 256], BF16, kind="ExternalInput")
    g2a = nc.dram_tensor("g2a", [768, 256], BF16, kind="ExternalInput")
    g1b = nc.dram_tensor("g1b", [256, 256], BF16, kind="ExternalInput")
    g2b = nc.dram_tensor("g2b", [256, 256], BF16, kind="ExternalInput")
    g1b1f = nc.dram_tensor("g1b1f", [128, 2], F32, kind="ExternalInput")
    g2b1f = nc.dram_tensor("g2b1f", [128, 2], F32, kind="ExternalInput")
    g1w2 = nc.dram_tensor("g1w2", [256, 256], BF16, kind="ExternalInput")
    g2w2 = nc.dram_tensor("g2w2", [256, 256], BF16, kind="ExternalInput")
    b2f4 = nc.dram_tensor("b2f4", [4, 128], BF16, kind="ExternalInput")
    e4sel = nc.dram_tensor("e4sel", [4, 128], BF16, kind="ExternalInput")
    ow1 = nc.dram_tensor("ow1", [640, 256], BF16, kind="ExternalInput")
    ob1 = nc.dram_tensor("ob1", [128, 2], F32, kind="ExternalInput")
    ow2 = nc.dram_tensor("ow2", [256, 1], BF16, kind="ExternalInput")
    ob2 = nc.dram_tensor("ob2", [1, 1], F32, kind="ExternalInput")

    out_d = nc.dram_tensor("out", [B, 1], F32, kind="ExternalOutput")

    import contextlib
    with tile.TileContext(nc) as tc:
        ctx = contextlib.ExitStack()
        with ctx:
            wpool = ctx.enter_context(tc.tile_pool(name="weights", bufs=1))
            hpool = ctx.enter_context(tc.tile_pool(name="hstate", bufs=2))
            rpool = ctx.enter_context(tc.tile_pool(name="rings", bufs=2))
            cpool = ctx.enter_context(tc.tile_pool(name="cring", bufs=3))
            spool = ctx.enter_context(tc.tile_pool(name="scratch", bufs=2))
            mpool = ctx.enter_context(tc.tile_pool(name="memstate", bufs=2))
            # PSUM: exactly 8 banks
            pacc = ctx.enter_context(tc.tile_pool(name="pacc", bufs=2, space="PSUM"))
            pst1 = ctx.enter_context(tc.tile_pool(name="pst1", bufs=1, space="PSUM"))
            pout = ctx.enter_context(tc.tile_pool(name="pout", bufs=1, space="PSUM"))
            psf = ctx.enter_context(tc.tile_pool(name="psf", bufs=1, space="PSUM"))
            p1g = ctx.enter_context(tc.tile_pool(name="p1g", bufs=1, space="PSUM"))
            p3g = ctx.enter_context(tc.tile_pool(name="p3g", bufs=2, space="PSUM"))

            # ---- resident weights / constants ----
            wihT_t = wpool.tile([128, 4, 1536], BF16)
            nc.sync.dma_start(
                wihT_t[:], waug.ap().rearrange("(kc p) c -> p kc c", p=128))
            whhT_t = wpool.tile([128, 1536], BF16)
            nc.sync.dma_start(whhT_t[:], whhT.ap())
            id_t = wpool.tile([128, 128], BF16)
            nc.sync.dma_start(id_t[:], ident.ap())
            ones128_t = wpool.tile([128, 1], BF16)
            nc.sync.dma_start(ones128_t[:], ones128.ap())
            ones1x128_t = wpool.tile([1, 128], BF16)
            nc.vector.memset(ones1x128_t[:], 1.0)

            a1w1_t = wpool.tile([128, 6, 256], BF16)
            nc.sync.dma_start(a1w1_t[:], a1w1.ap().rearrange("(kc p) c -> p kc c", p=128))
            a1b1_t = wpool.tile([128, 2], F32)
            nc.sync.dma_start(a1b1_t[:], a1b1.ap())
            a1w2_t = wpool.tile([128, 2, 768], BF16)
            nc.sync.dma_start(a1w2_t[:], a1w2.ap().rearrange("(kc p) c -> p kc c", p=128))
            a1b2_t = wpool.tile([128, 6], F32)
            nc.sync.dma_start(a1b2_t[:], a1b2.ap())
            a2w1_t = wpool.tile([128, 6, 256], BF16)
            nc.sync.dma_start(a2w1_t[:], a2w1.ap().rearrange("(kc p) c -> p kc c", p=128))
            a2b1f_t = wpool.tile([128, 2], F32)
            nc.sync.dma_start(a2b1f_t[:], a2b1f.ap())
            a2w2_t = wpool.tile([128, 2, 256], BF16)
            nc.sync.dma_start(a2w2_t[:], a2w2.ap().rearrange("(kc p) c -> p kc c", p=128))
            a2b2f_t = wpool.tile([128, 2], F32)
            nc.sync.dma_start(a2b2f_t[:], a2b2f.ap())
            g1a_t = wpool.tile([128, 6, 256], BF16)
            nc.sync.dma_start(g1a_t[:], g1a.ap().rearrange("(kc p) c -> p kc c", p=128))
            g2a_t = wpool.tile([128, 6, 256], BF16)
            nc.sync.dma_start(g2a_t[:], g2a.ap().rearrange("(kc p) c -> p kc c", p=128))
            g1b1f_t = wpool.tile([128, 2], F32)
            nc.sync.dma_start(g1b1f_t[:], g1b1f.ap())
            g2b1f_t = wpool.tile([128, 2], F32)
            nc.sync.dma_start(g2b1f_t[:], g2b1f.ap())
            g1b_t = wpool.tile([128, 2, 256], BF16)
            nc.sync.dma_start(g1b_t[:], g1b.ap().rearrange("(kc p) c -> p kc c", p=128))
            g2b_t = wpool.tile([128, 2, 256], BF16)
            nc.sync.dma_start(g2b_t[:], g2b.ap().rearrange("(kc p) c -> p kc c", p=128))
            g1w2_t = wpool.tile([128, 2, 256], BF16)
            nc.sync.dma_start(g1w2_t[:], g1w2.ap().rearrange("(kc p) c -> p kc c", p=128))
            g2w2_t = wpool.tile([128, 2, 256], BF16)
            nc.sync.dma_start(g2w2_t[:], g2w2.ap().rearrange("(kc p) c -> p kc c", p=128))
            b2f4_t = wpool.tile([4, 128], BF16)
            nc.sync.dma_start(b2f4_t[:], b2f4.ap())
            e4sel_t = wpool.tile([4, 128], BF16)
            nc.sync.dma_start(e4sel_t[:], e4sel.ap())
            ow1_t = wpool.tile([128, 5, 256], BF16)
            nc.sync.dma_start(ow1_t[:], ow1.ap().rearrange("(kc p) c -> p kc c", p=128))
            ob1_t = wpool.tile([128, 2], F32)
            nc.sync.dma_start(ob1_t[:], ob1.ap())
            ow2_t = wpool.tile([128, 2, 1], BF16)
            nc.sync.dma_start(ow2_t[:], ow2.ap().rearrange("(kc p) c -> p kc c", p=128))
            ob2_t = wpool.tile([1, 1], F32)
            nc.sync.dma_start(ob2_t[:], ob2.ap())

            # ---- persistent state ----
            st = {}
            st["h"] = hpool.tile([128, 96], BF16, tag="h", name="h0")
            nc.vector.memset(st["h"][:], 0.0)
            st["c"] = hpool.tile([128, 3, 32], F32, tag="c", name="c0")
            nc.vector.memset(st["c"][:], 0.0)
            st["mem"] = mpool.tile([128, 2, 32], BF16, tag="mem", name="mem0")
            nc.vector.memset(st["mem"][:], 0.0)

            # ring tile dicts keyed by chunk
            WIN, CB, ET, SRB, CHS, GS = {}, {}, {}, {}, {}, {}

            # ---------------- emission units ----------------
            def ph0_units(k):
                t0 = k * CH
                units = []
                xt = spool.tile([128, 4, NB2], BF16, tag="xt", name=f"xt{k}")
                win = rpool.tile([128, 12, CH, 32], BF16, tag="win", name=f"win{k}")
                WIN[k] = win

                def load():
                    for kc in range(4):
                        rows = 128 if kc < 3 else DAUG - 384
                        nc.sync.dma_start(
                            xt[0:rows, kc, :],
                            xT.ap()[kc * 128:kc * 128 + rows, t0 * B:(t0 + CH) * B])
                units.append(load)

                def slot(s):
                    def f():
                        pt = pacc.tile([128, NB2], F32, tag="acc", name=f"p0a{k}_{s}")
                        kcs = _nonzero_kcs(s)
                        for i, kc in enumerate(kcs):
                            rows = 128 if kc < 3 else DAUG - 384
                            nc.tensor.matmul(
                                pt[:], wihT_t[0:rows, kc, s * 128:(s + 1) * 128],
                                xt[0:rows, kc, :],
                                start=(i == 0), stop=(i == len(kcs) - 1))
                        dst = win[:, s, :, :]
                        if s % 2 == 0:
                            nc.vector.tensor_copy(dst, pt[:].rearrange("p (t b) -> p t b", b=B))
                        else:
                            nc.scalar.copy(dst, pt[:].rearrange("p (t b) -> p t b", b=B))
                    return f
                units += [slot(s) for s in range(12)]
                return units

            def ph1_step(k, j):
                win = WIN[k]

                def f():
                    if j == 0:
                        cb = cpool.tile([128, 3, CH + 1, 32], BF16, tag="cb", name=f"cb{k}")
                        CB[k] = cb
                        nc.gpsimd.tensor_copy(cb[:, :, 0, :], st["c"][:])
                    cb = CB[k]
                    gp = p1g.tile([128, 12, 32], F32, tag="gates", name=f"g1p{k}_{j}")
                    nc.tensor.matmul(gp[:], id_t[:], win[:, :, j, :],
                                     start=True, stop=False)
                    for s in range(12):
                        nc.tensor.matmul(
                            gp[:, s, :], whhT_t[:, s * 128:(s + 1) * 128],
                            st["h"][:, (s % 3) * 32:(s % 3 + 1) * 32],
                            start=False, stop=(s == 11))
                    tg = spool.tile([128, 3, 32], F32, tag="tg", name=f"tg{k}_{j}")
                    nc.scalar.activation(tg[:], gp[:, 9:12, :], AF.Tanh)
                    # sigmoid(x) = 0.5*tanh(x/2)+0.5 (keeps ACT in the exp table set)
                    th9 = spool.tile([128, 9, 32], F32, tag="th9", name=f"th{k}_{j}")
                    nc.scalar.activation(th9[:], gp[:, 0:9, :], AF.Tanh, scale=0.5)
                    sg = spool.tile([128, 6, 32], F32, tag="sg", name=f"sg{k}_{j}")
                    nc.vector.tensor_scalar(sg[:], th9[:, 0:6, :], 0.5, 0.5,
                                            mybir.AluOpType.mult, mybir.AluOpType.add)
                    so = spool.tile([128, 3, 32], F32, tag="so", name=f"so{k}_{j}")
                    nc.gpsimd.tensor_scalar(so[:], th9[:, 6:9, :], 0.5, 0.5,
                                            mybir.AluOpType.mult, mybir.AluOpType.add)
                    t1 = spool.tile([128, 3, 32], F32, tag="t1", name=f"t1{k}_{j}")
                    nc.gpsimd.tensor_mul(t1[:], sg[:, 0:3, :], tg[:])
                    t2 = spool.tile([128, 3, 32], F32, tag="t2", name=f"t2{k}_{j}")
                    nc.vector.tensor_mul(t2[:], sg[:, 3:6, :], st["c"][:])
                    c_new = hpool.tile([128, 3, 32], F32, tag="c", name=f"c{k}_{j}")
                    nc.vector.tensor_add(c_new[:], t1[:], t2[:])
                    nc.gpsimd.tensor_copy(cb[:, :, j + 1, :], c_new[:])
                    tc_t = spool.tile([128, 3, 32], F32, tag="tc", name=f"tc{k}_{j}")
                    nc.scalar.activation(tc_t[:], c_new[:], AF.Tanh)
                    h_new = hpool.tile([128, 96], BF16, tag="h", name=f"h{k}_{j}")
                    nc.vector.tensor_mul(
                        h_new[:].rearrange("p (m b) -> p m b", b=32),
                        so[:], tc_t[:])
                    st["c"] = c_new
                    st["h"] = h_new
                return f

            def alpha_units(k):
                units = []
                cb = CB[k]

                def rhs_k(kc):
                    if kc < 3:
                        return cb[:, kc, 0:CH, :]
                    return cb[:, kc - 3, 1:CH + 1, :]

                y1p = pst1.tile([128, 2, NB2], F32, tag="stage1", name=f"y1p{k}")
                y1 = spool.tile([128, 2, NB2], BF16, tag="y1", name=f"y1{k}")

                def att1_l1(mc):
                    def f():
                        for kc in range(6):
                            nc.tensor.matmul(
                                y1p[:, mc, :], a1w1_t[:, kc, mc * 128:(mc + 1) * 128],
                                rhs_k(kc), start=(kc == 0), stop=(kc == 5))
                        nc.scalar.activation(y1[:, mc, :], y1p[:, mc, :], AF.Relu,
                                             bias=a1b1_t[:, mc:mc + 1])
                    return f
                units += [att1_l1(0), att1_l1(1)]

                et = rpool.tile([128, 6, NB2], BF16, tag="et", name=f"et{k}")
                ET[k] = et

                def logit(mc6):
                    def f():
                        ep = pacc.tile([128, NB2], F32, tag="acc", name=f"ep{k}_{mc6}")
                        for kc in range(2):
                            nc.tensor.matmul(
                                ep[:], a1w2_t[:, kc, mc6 * 128:(mc6 + 1) * 128],
                                y1[:, kc, :], start=(kc == 0), stop=(kc == 1))
                        nc.scalar.activation(et[:, mc6, :], ep[:], AF.Exp,
                                             bias=a1b2_t[:, mc6:mc6 + 1])
                    return f
                units += [logit(m6) for m6 in range(6)]

                def ssum():
                    sp = psf.tile([1, NB2], F32, tag="srow", name=f"sp{k}")
                    for q in range(6):
                        nc.tensor.matmul(sp[:], ones128_t[:], et[:, q, :],
                                         start=(q == 0), stop=(q == 5))
                    srec = spool.tile([1, NB2], F32, tag="srec", name=f"sr{k}")
                    nc.vector.reciprocal(srec[:], sp[:])
                    srb = rpool.tile([1, NB2], BF16, tag="srb", name=f"srb{k}")
                    SRB[k] = srb
                    nc.gpsimd.tensor_copy(srb[:], srec[:])
                units.append(ssum)
                return units

            def beta_units(k):
                units = []
                et = ET[k]
                cb = CB[k]
                sbb = spool.tile([128, NB2], BF16, tag="sbb", name=f"sbb{k}")
                etn = spool.tile([128, 6, NB2], BF16, tag="etn", name=f"etn{k}")
                at = rpool.tile([128, 6, NB2], BF16, tag="at", name=f"at{k}")

                def bpre():
                    sbp = pout.tile([128, 2, NB2], F32, tag="out", name=f"sbp{k}")
                    nc.tensor.matmul(sbp[:, 0, :], ones1x128_t[:], SRB[k][:],
                                     start=True, stop=True)
                    nc.scalar.copy(sbb[:], sbp[:, 0, :])
                    nc.vector.tensor_mul(
                        etn[:], et[:], sbb[:].unsqueeze(1).broadcast_to([128, 6, NB2]))
                    nc.vector.tensor_mul(
                        at[:, 0:3, :].rearrange("p q (t b) -> p q t b", b=B),
                        etn[:, 0:3, :].rearrange("p q (t b) -> p q t b", b=B),
                        cb[:, :, 0:CH, :])
                    nc.vector.tensor_mul(
                        at[:, 3:6, :].rearrange("p q (t b) -> p q t b", b=B),
                        etn[:, 3:6, :].rearrange("p q (t b) -> p q t b", b=B),
                        cb[:, :, 1:CH + 1, :])
                units.append(bpre)

                zp = pst1.tile([128, 2, NB2], F32, tag="stage1", name=f"zp{k}")
                z = spool.tile([128, 2, NB2], BF16, tag="z", name=f"z{k}")

                def att2_l1(mc):
                    def f():
                        for kc in range(6):
                            nc.tensor.matmul(
                                zp[:, mc, :], a2w1_t[:, kc, mc * 128:(mc + 1) * 128],
                                at[:, kc, :], start=(kc == 0), stop=(kc == 5))
                        nc.scalar.activation(z[:, mc, :], zp[:, mc, :], AF.Relu,
                                             bias=a2b1f_t[:, mc:mc + 1])
                    return f
                units += [att2_l1(0), att2_l1(1)]

                chs = rpool.tile([128, 2, CH, 32], BF16, tag="chs", name=f"chs{k}")
                CHS[k] = chs

                def att2_l2():
                    ap2 = pout.tile([128, 2, NB2], F32, tag="out", name=f"ap2{k}")
                    for mc in range(2):
                        for kc in range(2):
                            nc.tensor.matmul(
                                ap2[:, mc, :], a2w2_t[:, kc, mc * 128:(mc + 1) * 128],
                                z[:, kc, :], start=(kc == 0), stop=(kc == 1))
                    for mc in range(2):
                        nc.scalar.activation(
                            chs[:, mc, :, :],
                            ap2[:, mc, :].rearrange("p (t b) -> p t b", b=B),
                            AF.Tanh, bias=a2b2f_t[:, mc:mc + 1])
                units.append(att2_l2)

                gs = rpool.tile([128, 4, CH, 32], BF16, tag="gs", name=f"gs{k}")
                GS[k] = gs

                def gpart(gi):
                    gw, gbf = ((g1a_t, g1b1f_t), (g2a_t, g2b1f_t))[gi]

                    def f():
                        gp2 = pout.tile([128, 2, NB2], F32, tag="out", name=f"gp2{k}_{gi}")
                        for mc in range(2):
                            for kc in range(6):
                                nc.tensor.matmul(
                                    gp2[:, mc, :], gw[:, kc, mc * 128:(mc + 1) * 128],
                                    at[:, kc, :], start=(kc == 0), stop=(kc == 5))
                        for mc in range(2):
                            dst = gs[:, gi * 2 + mc, :, :]
                            src = gp2[:, mc, :].rearrange("p (t b) -> p t b", b=B)
                            if gi == 0:
                                nc.vector.tensor_scalar_add(dst, src, gbf[:, mc:mc + 1])
                            else:
                                nc.scalar.activation(dst, src, AF.Identity,
                                                     bias=gbf[:, mc:mc + 1])
                    return f
                units += [gpart(0), gpart(1)]
                return units

            def ph3_step(k, j):
                def f():
                    gw_t, chw = GS[k], CHS[k]
                    pq = p3g.tile([128, 8, 32], F32, tag="g3", name=f"pq{k}_{j}")
                    pg = pq[:, 0:4, :]
                    qg = pq[:, 4:8, :]
                    # L2 b2 inject first (independent of this step's deps)
                    nc.tensor.matmul(qg, b2f4_t[:], e4sel_t[:], start=True, stop=False)
                    # L1: att-part inject + mem matmuls
                    nc.tensor.matmul(pg, id_t[:], gw_t[:, :, j, :],
                                     start=True, stop=False)
                    for r, gwt in enumerate((g1b_t, g1b_t, g2b_t, g2b_t)):
                        mc = r % 2
                        for kc in range(2):
                            nc.tensor.matmul(
                                pq[:, r, :], gwt[:, kc, mc * 128:(mc + 1) * 128],
                                st["mem"][:, kc, :], start=False,
                                stop=(r == 3 and kc == 1))
                    hh = spool.tile([128, 4, 32], BF16, tag="hh", name=f"hh{k}_{j}")
                    nc.scalar.activation(hh[:], pg, AF.Relu)
                    for r, gwt in enumerate((g1w2_t, g1w2_t, g2w2_t, g2w2_t)):
                        mc = r % 2
                        goff = 0 if r < 2 else 2
                        for kc in range(2):
                            nc.tensor.matmul(
                                pq[:, 4 + r, :], gwt[:, kc, mc * 128:(mc + 1) * 128],
                                hh[:, goff + kc, :], start=False,
                                stop=(r == 3 and kc == 1))
                    th4 = spool.tile([128, 4, 32], F32, tag="th4", name=f"th4{k}_{j}")
                    nc.scalar.activation(th4[:], qg, AF.Tanh, scale=0.5)
                    gam = spool.tile([128, 4, 32], BF16, tag="gam", name=f"gam{k}_{j}")
                    nc.gpsimd.tensor_scalar(gam[:], th4[:], 0.5, 0.5,
                                            mybir.AluOpType.mult, mybir.AluOpType.add)
                    m1 = spool.tile([128, 2, 32], BF16, tag="m1", name=f"m1{k}_{j}")
                    nc.gpsimd.tensor_mul(m1[:], gam[:, 0:2, :], st["mem"][:])
                    m2 = spool.tile([128, 2, 32], BF16, tag="m2", name=f"m2{k}_{j}")
                    nc.vector.tensor_mul(m2[:], gam[:, 2:4, :], chw[:, :, j, :])
                    mem_new = mpool.tile([128, 2, 32], BF16, tag="mem", name=f"mem{k}_{j}")
                    nc.vector.tensor_add(mem_new[:], m1[:], m2[:])
                    st["mem"] = mem_new
                return f

            # ---------------- fused pipeline loop ----------------
            NITER = NCH + 4
            for i in range(NITER):
                fillers = []
                if i < NCH:
                    fillers += ph0_units(i)
                if 0 <= i - 2 < NCH:
                    fillers += alpha_units(i - 2)
                if 0 <= i - 3 < NCH:
                    fillers += beta_units(i - 3)
                steps = []
                if 0 <= i - 1 < NCH:
                    steps.append([ph1_step(i - 1, j) for j in range(CH)])
                if 0 <= i - 4 < NCH:
                    steps.append([ph3_step(i - 4, j) for j in range(CH)])
                # round-robin: serial steps first, fillers spread between
                nf = len(fillers)
                fi = 0
                for j in range(CH):
                    for sl in steps:
                        sl[j]()
                    take = (nf * (j + 1)) // CH - (nf * j) // CH
                    for _ in range(take):
                        fillers[fi]()
                        fi += 1
                assert fi == nf

            # ---------------- PHASE 4: output MLP ----------------
            if True:
                h_fin = st["h"]
                mem_fin = st["mem"]
                o1p = p3g.tile([128, 2, 32], F32, tag="g3", name="o1p")
                rhs5 = [h_fin[:, 0:32], h_fin[:, 32:64], h_fin[:, 64:96],
                        mem_fin[:, 0, :], mem_fin[:, 1, :]]
                for mc in range(2):
                    for kc in range(5):
                        nc.tensor.matmul(
                            o1p[:, mc, :], ow1_t[:, kc, mc * 128:(mc + 1) * 128],
                            rhs5[kc], start=(kc == 0), stop=(kc == 4))
                o1s = spool.tile([128, 2, 32], BF16, tag="o1s")
                for mc in range(2):
                    nc.scalar.activation(o1s[:, mc, :], o1p[:, mc, :], AF.Relu,
                                         bias=ob1_t[:, mc:mc + 1])
                o2p = psf.tile([1, 32], F32, tag="srow", name="o2p")
                for kc in range(2):
                    nc.tensor.matmul(o2p[:], ow2_t[:, kc, :], o1s[:, kc, :],
                                     start=(kc == 0), stop=(kc == 1))
                o2s = spool.tile([1, 32], F32, tag="o2s")
                nc.scalar.activation(o2s[:], o2p[:], AF.Identity, bias=ob2_t[:])
                nc.sync.dma_start(out_d.ap().rearrange("b one -> (one) (b)"), o2s[:])

    nc.compile()
    return nc


# ---------------------------------------------------------------------------
# host-side packing
# ---------------------------------------------------------------------------

def pack_shared(inp):
    f = np.float32
    d = {}
    wih = {0: inp["Wih_l"], 1: inp["Wih_a"], 2: inp["Wih_v"]}
    whh = {0: inp["Whh_l"], 1: inp["Whh_a"], 2: inp["Whh_v"]}
    bb = {m: (inp[f"bih_{k}"] + inp[f"bhh_{k}"]).astype(f)
          for m, k in ((0, "l"), (1, "a"), (2, "v"))}
    foff = {0: 0, 1: D_L, 2: D_L + D_A}
    din = {0: D_L, 1: D_A, 2: D_V}

    waug = np.zeros((512, 1536), f)
    whhT = np.zeros((128, 1536), f)
    for gq in range(4):
        tg = TORCH_G[gq]
        for m in range(3):
            s = gq * 3 + m
            wblk = wih[m][tg * 128:(tg + 1) * 128, :]
            waug[foff[m]:foff[m] + din[m], s * 128:(s + 1) * 128] = wblk.T
            waug[DIN, s * 128:(s + 1) * 128] = bb[m][tg * 128:(tg + 1) * 128]
            whhT[:, s * 128:(s + 1) * 128] = whh[m][tg * 128:(tg + 1) * 128, :].T
    d["waug"] = waug.astype(NPBF)
    d["whhT"] = whhT.astype(NPBF)
    d["ident"] = np.eye(128, dtype=f).astype(NPBF)
    d["ones128"] = np.ones((128, 1), f).astype(NPBF)

    bf = lambda a: np.ascontiguousarray(np.asarray(a, f)).astype(NPBF)
    fm2 = lambda b: np.ascontiguousarray(np.asarray(b, f).reshape(2, 128).T)

    d["a1w1"] = bf(np.asarray(inp["att1_W1"]).T)
    d["a1b1"] = fm2(inp["att1_b1"])
    d["a1w2"] = bf(np.asarray(inp["att1_W2"]).T)
    d["a1b2"] = np.ascontiguousarray(np.asarray(inp["att1_b2"], f).reshape(6, 128).T)
    d["a2w1"] = bf(np.asarray(inp["att2_W1"]).T)
    d["a2b1f"] = fm2(inp["att2_b1"])
    d["a2w2"] = bf(np.asarray(inp["att2_W2"]).T)
    d["a2b2f"] = fm2(inp["att2_b2"])
    d["g1a"] = bf(np.asarray(inp["g1_W1"])[:, :768].T)
    d["g2a"] = bf(np.asarray(inp["g2_W1"])[:, :768].T)
    d["g1b"] = bf(np.asarray(inp["g1_W1"])[:, 768:].T)
    d["g2b"] = bf(np.asarray(inp["g2_W1"])[:, 768:].T)
    d["g1b1f"] = fm2(inp["g1_b1"])
    d["g2b1f"] = fm2(inp["g2_b1"])
    d["g1w2"] = bf(np.asarray(inp["g1_W2"]).T)
    d["g2w2"] = bf(np.asarray(inp["g2_W2"]).T)
    d["b2f4"] = bf(np.concatenate([np.asarray(inp["g1_b2"]),
                                   np.asarray(inp["g2_b2"])]).reshape(4, 128))
    d["e4sel"] = bf(np.kron(np.eye(4, dtype=f), np.ones((1, 32), f)))
    d["ow1"] = bf(np.asarray(inp["out_W1"]).T)
    d["ob1"] = fm2(inp["out_b1"])
    d["ow2"] = bf(np.asarray(inp["out_W2"]).T)
    d["ob2"] = np.asarray(inp["out_b2"], f).reshape(1, 1).copy()
    return d


def pack_x(x, core, Tp):
    xc = np.asarray(x[:, core * B:(core + 1) * B, :], np.float32)
    xt = xc.transpose(2, 0, 1).reshape(DIN, Tp * B)
    return np.concatenate([xt, np.ones((1, Tp * B), np.float32)], 0).astype(NPBF)


_CACHE = {}


def _get_program(Tp):
    if Tp not in _CACHE:
        _CACHE[Tp] = build_program(Tp)
    return _CACHE[Tp]


def kernel(**inputs):
    x = np.asarray(inputs["x"])
    Tp = x.shape[0]
    nc = _get_program(Tp)
    shared = pack_shared({k: np.asarray(v) for k, v in inputs.items()})
    in_maps = []
    for c in range(NCORES):
        m = dict(shared)
        m["xT"] = np.ascontiguousarray(pack_x(x, c, Tp))
        in_maps.append(m)
    res = run_bass_kernel_spmd(nc, in_maps, list(range(NCORES))).results
    out = np.concatenate([r["out"] for r in res], axis=0)
    return out.astype(np.float32)


if __name__ == "__main__":
    import time
    t0 = time.time()
    nc = build_program(64)
    print("built in", time.time() - t0, "s")
